# revision 36
# baseline (speedup 1.0000x reference)
"""Trainium2 Bass kernel for the A3TGCN-2-points model (8 NeuronCores, data-parallel).

Math (verified vs a line-by-line port of the reference at 3.5e-8):
  - The reference passes H=None each period, so H0 = 0: the reset gate R
    vanishes and only the first HID rows of L_z / L_h matter.
  - x_temporal takes two values per sample (admission cols before t < LOS,
    discharge after), so the 37-step attention scan collapses to
        H = w * cell(ad) + (1 - w) * cell(dis),  w = cumsum(softmax(att))[LOS]
    cell(X) = (1 - sigmoid(A X Wz Lz1 + beta_z)) * tanh(A X Wh Lh1 + beta_h)
  - 1 - sigmoid(s) = (1 - tanh(s/2)) / 2, so ONE tanh evaluates both gates
    (the z columns of the fused weights are scaled by -1/2).

Sharding: batch 512 -> 64 samples per core; all weights replicated; the
[64, 1] logits per core are concatenated on the host.

Device mapping, default version 2 (BASSKERNEL_VER=1 selects the dma_gather
variant):
  - embedding lookup via PE one-hot matmuls from a host-transposed [v,(c,e)]
    table (one [100,32]x[100,64] matmul per column) -> X^T, e on partitions
  - fused gate weights Mzh = [-Wz@Lz1/2 | Wh@Lh1] applied as one matmul per
    512 columns -> Q^T, then a (c,b)->(b,c) free-axis reorder (split across
    ACT/GPSIMD/DVE - it costs ~2.5us/512 cols on any single engine)
  - per 128-row chunk: PE transpose, adjacency I4 (x) A^T matmul (gcn_norm
    built on device from edge_index via one-hot matmuls), bias add, tanh,
    gate combine, and pooling+LOS-blend fused into a [128,64]x[128,2] matmul
  - classifier MLP on the pooled [64, 64] tile, 64 f32 out per core.
"""

import os
import sys

import numpy as np

sys.path.insert(0, "/opt/trn_rl_repo")

import concourse.bacc as bacc
import concourse.bass as bass
import concourse.mybir as mybir
import concourse.tile as tile
from concourse.bass_utils import run_bass_kernel_spmd

F32 = mybir.dt.float32
I32 = mybir.dt.int32
I16 = mybir.dt.int16
I8 = mybir.dt.int8
BF16 = mybir.dt.bfloat16
AF = mybir.ActivationFunctionType
ALU = mybir.AluOpType

B, C, N, V, EMB, HID, E, T = 512, 64, 32, 100, 32, 64, 256, 37
NCORES = 8
BSH = B // NCORES            # samples per core
R = BSH * C                  # gathered rows per core (4096)
NCHUNK = R // 128            # 32 row-chunks of 128
NBATCH = NCHUNK // 4         # 8 batches of 4 chunks ([*, 512] tiles)

# blob column layout (weights packed into one [128, 392] f32 DMA)
OWZ, OWH, OLZ, OLH, OWC1 = 0, 64, 128, 192, 256
OBZ, OBH, OLBZ, OLBH, OWC2, OBC1, OBC2, OATT = 384, 385, 386, 387, 388, 389, 390, 391
OID, OION, OIOT = 392, 520, 552
OI100, OPDIF, OPDIS = 553, 554, 618
BLOBF = 682


def _install_ntff_hook():
    """The agent image's antenv lacks axon_hooks; synthesize it so trace=True
    can drive NTFF profiling via ctypes on libaxon_pjrt.so (mirrors the
    boot-side hook in trn_boot.py)."""
    import contextlib
    import ctypes
    import types

    if "antenv.axon_hooks" in sys.modules:
        return
    so_path = "/opt/axon/libaxon_pjrt.so"
    mod = types.ModuleType("antenv.axon_hooks")
    state = {"hook": None}

    def set_axon_ntff_profile_hook(h):
        state["hook"] = h

    def get_axon_ntff_profile_hook():
        return state["hook"]

    mod.set_axon_ntff_profile_hook = set_axon_ntff_profile_hook
    mod.get_axon_ntff_profile_hook = get_axon_ntff_profile_hook
    sys.modules["antenv.axon_hooks"] = mod
    try:
        import antenv
        antenv.axon_hooks = mod
    except ImportError:
        pass

    if not os.path.exists(so_path):
        return
    lib = ctypes.CDLL(so_path)
    if not hasattr(lib, "axon_start_nrt_profile"):
        return
    lib.axon_start_nrt_profile.argtypes = [ctypes.POINTER(ctypes.c_int64), ctypes.c_size_t]
    lib.axon_start_nrt_profile.restype = ctypes.c_int64
    lib.axon_stop_nrt_profile.argtypes = [ctypes.c_char_p]
    lib.axon_stop_nrt_profile.restype = ctypes.c_int64

    @contextlib.contextmanager
    def _hook(output_dir, device_ids):
        import jax
        jax.devices()
        if device_ids:
            ids = (ctypes.c_int64 * len(device_ids))(*device_ids)
            rc = lib.axon_start_nrt_profile(ids, len(device_ids))
        else:
            rc = lib.axon_start_nrt_profile(None, 0)
        if rc != 0:
            raise RuntimeError(f"axon_start_nrt_profile rc={rc}")
        try:
            yield
        finally:
            n = lib.axon_stop_nrt_profile(str(output_dir).encode())
            print(f"profile: {n} file(s) written to {output_dir}", file=sys.stderr)

    set_axon_ntff_profile_hook(_hook)


_CACHE = {}
LAST_EXEC_NS = None


def _build_nc():
    nc = bacc.Bacc("TRN2")

    tp = nc.declare_dram_parameter("tp", [C * V, 64], F32, isOutput=False)
    gidx = nc.declare_dram_parameter("gidx", [128, R // 16], I16, isOutput=False)
    edge = nc.declare_dram_parameter("edge", [2, E], I32, isOutput=False)
    los = nc.declare_dram_parameter("los", [1, BSH], I32, isOutput=False)
    blob = nc.declare_dram_parameter("blob", [128, BLOBF], F32, isOutput=False)
    out = nc.declare_dram_parameter("out", [1, BSH], F32, isOutput=True)

    with tile.TileContext(nc) as tc:
        with (
            tc.tile_pool(name="const", bufs=1) as cp,
            tc.tile_pool(name="work", bufs=3) as wp,
            tc.tile_pool(name="ppY", bufs=3, space="PSUM") as ppY,
            tc.tile_pool(name="ppS", bufs=2, space="PSUM") as ppS,
            tc.tile_pool(name="ppA", bufs=1, space="PSUM") as ppA,
        ):
            # ---------------- input DMAs ----------------
            gsb = cp.tile([128, R // 16], I16)
            nc.sync.dma_start(out=gsb[:], in_=gidx[:])
            # dummy 16-row dma_gather: forces the Q7 mlp library load to
            # overlap the input DMAs instead of delaying the first real gather
            warm_idx = cp.tile([128, 1], I16)
            nc.vector.memset(warm_idx[:], 0)
            warm_out = cp.tile([128, 1, 64], F32)
            nc.gpsimd.dma_gather(
                out_ap=warm_out[:], in_ap=tp[:], idxs_ap=warm_idx[:],
                num_idxs=16, num_idxs_reg=16, elem_size=64)
            blob_sb = cp.tile([128, BLOBF], F32)
            nc.sync.dma_start(out=blob_sb[:], in_=blob[:])
            esrc = cp.tile([128, 2], I32)
            nc.sync.dma_start(out=esrc[:], in_=edge[0].rearrange("(k p) -> p k", p=128))
            edst = cp.tile([128, 2], I32)
            nc.sync.dma_start(out=edst[:], in_=edge[1].rearrange("(k p) -> p k", p=128))
            los_sb = cp.tile([1, BSH], I32)
            nc.sync.dma_start(out=los_sb[:], in_=los[:])

            def bcol(off, rows=64):
                return blob_sb[0:rows, off:off + 1]

            # ---------------- embedding gather ----------------
            # the SWDGE descriptor ring tops out between 1k and 2k entries per
            # shot; 4 gathers of 1024 rows, interleaved with the batches that
            # consume them (emitted in the main loop below)
            xg = cp.tile([128, NCHUNK, 64], F32)
            GCH = 1024

            def issue_gather(c0, c1):
                # gathers rows for chunks [c0, c1)
                nc.gpsimd.dma_gather(
                    out_ap=xg[:, c0:c1, :],
                    in_ap=tp[:],
                    idxs_ap=gsb[:, 8 * c0:8 * c1],
                    num_idxs=128 * (c1 - c0),
                    num_idxs_reg=128 * (c1 - c0),
                    elem_size=64,
                )

            # ---------------- constants ----------------
            id128 = blob_sb[:, OID:OID + 128]
            ones_col = cp.tile([128, 1], F32)
            nc.vector.memset(ones_col[:], 1.0)
            ones_row = cp.tile([1, 128], F32)
            nc.vector.memset(ones_row[:], 1.0)
            iota_nf = cp.tile([128, N], F32)
            _src = blob[0, OION:OION + N]
            nc.sync.dma_start(out=iota_nf[:], in_=bass.AP(_src.tensor, _src.offset, [[0, 128]] + list(_src.ap)))

            # ---------------- adjacency build: BD = I4 (x) A^T ----------------
            srcf = cp.tile([128, 2], F32)
            nc.vector.tensor_copy(srcf[:], esrc[:])
            dstf = cp.tile([128, 2], F32)
            nc.vector.tensor_copy(dstf[:], edst[:])

            Dk, Sk = [], []
            for k in range(2):
                d = cp.tile([128, N], F32, tag=f"dk{k}")
                nc.vector.tensor_tensor(
                    out=d[:], in0=dstf[:, k:k + 1].to_broadcast([128, N]),
                    in1=iota_nf[:], op=ALU.is_equal)
                s = cp.tile([128, N], F32, tag=f"sk{k}")
                nc.vector.tensor_tensor(
                    out=s[:], in0=srcf[:, k:k + 1].to_broadcast([128, N]),
                    in1=iota_nf[:], op=ALU.is_equal)
                Dk.append(d)
                Sk.append(s)

            deg_ps = ppS.tile([1, N], F32, tag="s_ps")
            nc.tensor.matmul(deg_ps[:], ones_col[:], Dk[0][:], start=True, stop=False)
            nc.tensor.matmul(deg_ps[:], ones_col[:], Dk[1][:], start=False, stop=True)
            degp1 = cp.tile([1, N], F32)
            nc.scalar.activation(degp1[:], deg_ps[:], AF.Identity, bias=1.0)
            rec = cp.tile([1, N], F32)
            nc.vector.reciprocal(rec[:], degp1[:])
            dinv_row = cp.tile([1, N], F32)
            nc.scalar.activation(dinv_row[:], rec[:], AF.Sqrt)

            dinvb_ps = ppS.tile([128, N], F32, tag="s_ps")
            nc.tensor.matmul(dinvb_ps[:], ones_row[:], dinv_row[:], start=True, stop=True)
            dinvb = cp.tile([128, N], F32)
            nc.vector.tensor_copy(dinvb[:], dinvb_ps[:])

            at_ps = ppA.tile([N, N], F32)
            for k in range(2):
                tmp = cp.tile([128, N], F32, tag="degtmp")
                nc.vector.tensor_tensor(out=tmp[:], in0=Dk[k][:], in1=dinvb[:], op=ALU.mult)
                dd = cp.tile([128, 1], F32, tag="ddk")
                nc.vector.tensor_reduce(dd[:], tmp[:], axis=mybir.AxisListType.X, op=ALU.add)
                nc.vector.tensor_tensor(out=tmp[:], in0=Sk[k][:], in1=dinvb[:], op=ALU.mult)
                ds_ = cp.tile([128, 1], F32, tag="dsk")
                nc.vector.tensor_reduce(ds_[:], tmp[:], axis=mybir.AxisListType.X, op=ALU.add)
                nrm = cp.tile([128, 1], F32, tag="nrmk")
                nc.vector.tensor_tensor(out=nrm[:], in0=dd[:], in1=ds_[:], op=ALU.mult)
                sn = cp.tile([128, N], F32, tag=f"snk{k}")
                nc.vector.tensor_scalar(out=sn[:], in0=Sk[k][:], scalar1=nrm[:, :1],
                                        scalar2=None, op0=ALU.mult)
                nc.tensor.matmul(at_ps[:], sn[:], Dk[k][:], start=(k == 0), stop=False)
            diagd = cp.tile([N, N], F32)
            nc.vector.tensor_tensor(out=diagd[:], in0=id128[:N, :N], in1=dinvb[:N, :],
                                    op=ALU.mult)
            nc.tensor.matmul(at_ps[:], diagd[:], diagd[:], start=False, stop=True)

            # engines are lane-locked (no partition shifts), so place the four
            # diagonal blocks with SBUF->SBUF DMAs
            at_sb = cp.tile([N, N], F32)
            nc.vector.tensor_copy(at_sb[:], at_ps[:])
            BD = cp.tile([128, 128], F32)
            nc.vector.memset(BD[:], 0.0)
            for q in range(4):
                nc.sync.dma_start(out=BD[32 * q:32 * (q + 1), 32 * q:32 * (q + 1)],
                                  in_=at_sb[:])

            # ---------------- fused gate weights Mzh = [-Mz/2 | Mh] ----------------
            mzh = cp.tile([EMB, 128], F32)
            betas = []
            for gi, (ow, ob, olb, olg, scale) in enumerate((
                    (OWZ, OBZ, OLBZ, OLZ, -0.5), (OWH, OBH, OLBH, OLH, 1.0))):
                wT_ps = ppS.tile([HID, EMB], F32, tag="s_ps")
                nc.tensor.transpose(wT_ps[:], blob_sb[0:EMB, ow:ow + HID], id128[:EMB, :EMB])
                wT = cp.tile([HID, EMB], F32, tag=f"wt{gi}")
                nc.vector.tensor_copy(wT[:], wT_ps[:])
                m_ps = ppS.tile([EMB, HID], F32, tag="s_ps")
                nc.tensor.matmul(m_ps[:], wT[:], blob_sb[0:HID, olg:olg + HID],
                                 start=True, stop=True)
                nc.scalar.activation(mzh[:, 64 * gi:64 * (gi + 1)], m_ps[:], AF.Copy,
                                     scale=scale)
                # beta_g = Lg1^T b_g + lb_g  (as a column), scaled like Mz/Mh
                bb_ps = ppS.tile([HID, 1], F32, tag="s_ps")
                nc.tensor.matmul(bb_ps[:], blob_sb[0:HID, olg:olg + HID], bcol(ob),
                                 start=True, stop=True)
                bsum = cp.tile([HID, 1], F32, tag=f"bsum{gi}")
                nc.vector.tensor_tensor(out=bsum[:], in0=bb_ps[:], in1=bcol(olb), op=ALU.add)
                bcolg = cp.tile([HID, 1], F32, tag=f"beta{gi}")
                nc.scalar.activation(bcolg[:], bsum[:], AF.Copy, scale=scale)
                betas.append(bcolg)

            # ---------------- LOS blend weights ----------------
            losf = cp.tile([1, BSH], F32)
            nc.vector.tensor_copy(losf[:], los_sb[:])
            losb_ps = ppS.tile([T, BSH], F32, tag="s_ps")
            nc.tensor.matmul(losb_ps[:], ones_row[:1, :T], losf[:], start=True, stop=True)
            mask = cp.tile([T, BSH], F32)
            nc.vector.tensor_tensor(out=mask[:], in0=blob_sb[0:T, OIOT:OIOT + 1].to_broadcast([T, BSH]),
                                    in1=losb_ps[:], op=ALU.is_lt)
            ecol = cp.tile([T, 1], F32)
            nc.scalar.activation(ecol[:], blob_sb[0:T, OATT:OATT + 1], AF.Exp)
            # preload the tanh ACT table here (idle window) so phase 2's first
            # tanh doesn't pay the 1.28us table swap on the critical path
            tanh_warm = cp.tile([1, 1], F32)
            nc.scalar.activation(tanh_warm[:], ones_col[0:1, 0:1], AF.Tanh)
            esum_ps = ppS.tile([1, 1], F32, tag="s_ps")
            nc.tensor.matmul(esum_ps[:], ecol[:], ones_col[:T, :], start=True, stop=True)
            rinv = cp.tile([1, 1], F32)
            nc.vector.reciprocal(rinv[:], esum_ps[:])
            wraw_ps = ppS.tile([1, BSH], F32, tag="s_ps")
            nc.tensor.matmul(wraw_ps[:], ecol[:], mask[:], start=True, stop=True)
            wrow = cp.tile([1, BSH], F32)
            nc.vector.tensor_scalar(out=wrow[:], in0=wraw_ps[:], scalar1=rinv[:, :1],
                                    scalar2=None, op0=ALU.mult)
            wb_ps = ppS.tile([HID, BSH], F32, tag="s_ps")
            nc.tensor.matmul(wb_ps[:], ones_row[:1, :HID], wrow[:], start=True, stop=True)
            wb = cp.tile([HID, BSH], F32)
            nc.vector.tensor_copy(wb[:], wb_ps[:])

            # ---------------- main loop ----------------
            sums = cp.tile([HID, 2 * BSH], F32)
            gather_plan = {0: (0, 8), 2: (8, 16), 4: (16, 24), 6: (24, 28), 7: (28, 32)}
            for jb in range(NBATCH):
                if jb in gather_plan:
                    issue_gather(*gather_plan[jb])
                y_ps = ppY.tile([EMB, 512], F32)
                for jj in range(4):
                    j = 4 * jb + jj
                    nc.tensor.matmul(y_ps[:, 128 * jj:128 * (jj + 1)],
                                     xg[:, j, 0:EMB], BD[:], start=True, stop=True)
                ysb = wp.tile([EMB, 512], F32)
                nc.vector.tensor_copy(ysb[:], y_ps[:])
                # z- and h-gate pre-activations side by side on the SAME
                # partitions (engines cannot shift lanes)
                s_ps = ppS.tile([HID, 1024], F32, tag="s_ps")
                nc.tensor.matmul(s_ps[:, 0:512], mzh[:, 0:64], ysb[:],
                                 start=True, stop=True)
                nc.tensor.matmul(s_ps[:, 512:1024], mzh[:, 64:128], ysb[:],
                                 start=True, stop=True)
                u = wp.tile([HID, 1024], BF16)
                nc.scalar.activation(u[:, 0:512], s_ps[:, 0:512], AF.Tanh,
                                     bias=betas[0][:, :1])
                nc.scalar.activation(u[:, 512:1024], s_ps[:, 512:1024], AF.Tanh,
                                     bias=betas[1][:, :1])
                w1 = wp.tile([HID, 512], BF16)
                nc.vector.tensor_tensor(out=w1[:], in0=u[:, 0:512], in1=u[:, 512:1024],
                                        op=ALU.mult)
                w2 = wp.tile([HID, 512], BF16)
                nc.vector.tensor_tensor(out=w2[:], in0=w1[:], in1=u[:, 512:1024],
                                        op=ALU.add)
                nc.vector.tensor_reduce(
                    sums[:, 16 * jb:16 * (jb + 1)],
                    w2[:].rearrange("p (g n) -> p g n", n=N),
                    axis=mybir.AxisListType.X, op=ALU.add)

            # ---------------- blend + pool + classifier ----------------
            s3 = sums[:].rearrange("p (s k) -> p s k", k=2)
            t1 = cp.tile([HID, BSH], F32)
            nc.vector.tensor_tensor(out=t1[:], in0=s3[:, :, 0], in1=s3[:, :, 1],
                                    op=ALU.subtract)
            t2 = cp.tile([HID, BSH], F32)
            nc.vector.tensor_tensor(out=t2[:], in0=t1[:], in1=wb[:], op=ALU.mult)
            pt = cp.tile([HID, BSH], F32)
            nc.vector.tensor_tensor(out=pt[:], in0=t2[:], in1=s3[:, :, 1], op=ALU.add)

            u1_ps = ppS.tile([2 * HID, BSH], F32, tag="s_ps")
            nc.tensor.matmul(u1_ps[:], blob_sb[0:HID, OWC1:OWC1 + 2 * HID], pt[:],
                             start=True, stop=True)
            v = cp.tile([2 * HID, BSH], F32)
            nc.scalar.activation(v[:], u1_ps[:], AF.Relu, bias=bcol(OBC1, 128),
                                 scale=1.0 / 64.0)
            y_ps2 = ppS.tile([1, BSH], F32, tag="s_ps")
            nc.tensor.matmul(y_ps2[:], blob_sb[0:128, OWC2:OWC2 + 1], v[:],
                             start=True, stop=True)
            yrow = cp.tile([1, BSH], F32)
            nc.scalar.activation(yrow[:], y_ps2[:], AF.Identity, bias=bcol(OBC2, 1))
            nc.sync.dma_start(out=out[:], in_=yrow[:])

    nc.finalize()
    return nc



def _build_nc_v2():
    """PE-one-hot variant: no GPSIMD at all (no library load, no descriptor
    prep).  Embedding lookup = per-column one-hot matmuls from a transposed
    bf16 table; adjacency applied on transposed row-chunks; pooling and the
    LOS blend fused into a per-chunk matmul."""
    nc = bacc.Bacc("TRN2")

    tp3 = nc.declare_dram_parameter("tp3", [V, C * EMB], BF16, isOutput=False)
    xbf = nc.declare_dram_parameter("xbf", [R], I8, isOutput=False)
    edge = nc.declare_dram_parameter("edge", [2, E], I32, isOutput=False)
    los = nc.declare_dram_parameter("los", [1, BSH], I32, isOutput=False)
    blob = nc.declare_dram_parameter("blob", [128, BLOBF], F32, isOutput=False)
    out = nc.declare_dram_parameter("out", [1, BSH], F32, isOutput=True)

    with tile.TileContext(nc) as tc:
        with (
            tc.tile_pool(name="const", bufs=1) as cp,
            tc.tile_pool(name="work", bufs=3) as wp,
            tc.tile_pool(name="pp1", bufs=3, space="PSUM") as pp1,
            tc.tile_pool(name="ppT", bufs=2, space="PSUM") as ppT,
            tc.tile_pool(name="ppS2", bufs=2, space="PSUM") as ppS2,
            tc.tile_pool(name="ppA", bufs=1, space="PSUM") as ppA,
        ):
            # ---------------- input DMAs ----------------
            blob_sb = cp.tile([128, BLOBF], F32)
            nc.sync.dma_start(out=blob_sb[:], in_=blob[:])
            tp3_sb = cp.tile([V, C * EMB], BF16)
            nc.sync.dma_start(out=tp3_sb[:], in_=tp3[:])
            esrc = cp.tile([128, 2], I32)
            nc.sync.dma_start(out=esrc[:], in_=edge[0].rearrange("(k p) -> p k", p=128))
            edst = cp.tile([128, 2], I32)
            nc.sync.dma_start(out=edst[:], in_=edge[1].rearrange("(k p) -> p k", p=128))
            los_sb = cp.tile([1, BSH], I32)
            nc.sync.dma_start(out=los_sb[:], in_=los[:])

            def bcol(off, rows=64):
                return blob_sb[0:rows, off:off + 1]

            id128 = blob_sb[:, OID:OID + 128]
            ones_col = cp.tile([128, 1], F32)
            nc.vector.memset(ones_col[:], 1.0)
            ones_row = cp.tile([1, 128], F32)
            nc.vector.memset(ones_row[:], 1.0)
            iota_nf = cp.tile([128, N], F32)
            _src = blob[0, OION:OION + N]
            nc.sync.dma_start(out=iota_nf[:], in_=bass.AP(_src.tensor, _src.offset, [[0, 128]] + list(_src.ap)))
            idb = cp.tile([128, 128], BF16)
            nc.scalar.activation(idb[:], id128, AF.Copy)

            # ---------------- adjacency: BD = I4 (x) A^T  (f32 + bf16) -----
            srcf = cp.tile([128, 2], F32)
            nc.vector.tensor_copy(srcf[:], esrc[:])
            dstf = cp.tile([128, 2], F32)
            nc.vector.tensor_copy(dstf[:], edst[:])
            Dk, Sk = [], []
            for k in range(2):
                d = cp.tile([128, N], F32, tag=f"dk{k}")
                nc.vector.tensor_tensor(out=d[:], in0=dstf[:, k:k + 1].to_broadcast([128, N]),
                                        in1=iota_nf[:], op=ALU.is_equal)
                s = cp.tile([128, N], F32, tag=f"sk{k}")
                nc.vector.tensor_tensor(out=s[:], in0=srcf[:, k:k + 1].to_broadcast([128, N]),
                                        in1=iota_nf[:], op=ALU.is_equal)
                Dk.append(d)
                Sk.append(s)
            deg_ps = pp1.tile([1, N], F32, tag="p1")
            nc.tensor.matmul(deg_ps[:], ones_col[:], Dk[0][:], start=True, stop=False)
            nc.tensor.matmul(deg_ps[:], ones_col[:], Dk[1][:], start=False, stop=True)
            degp1 = cp.tile([1, N], F32)
            nc.scalar.activation(degp1[:], deg_ps[:], AF.Identity, bias=1.0)
            rec = cp.tile([1, N], F32)
            nc.vector.reciprocal(rec[:], degp1[:])
            dinv_row = cp.tile([1, N], F32)
            nc.scalar.activation(dinv_row[:], rec[:], AF.Sqrt)
            dinvb_ps = pp1.tile([128, N], F32, tag="p1")
            nc.tensor.matmul(dinvb_ps[:], ones_row[:], dinv_row[:], start=True, stop=True)
            dinvb = cp.tile([128, N], F32)
            nc.vector.tensor_copy(dinvb[:], dinvb_ps[:])
            at_ps = ppA.tile([N, N], F32, tag="pA")
            for k in range(2):
                tmp = cp.tile([128, N], F32, tag="degtmp")
                nc.vector.tensor_tensor(out=tmp[:], in0=Dk[k][:], in1=dinvb[:], op=ALU.mult)
                dd = cp.tile([128, 1], F32, tag="ddk")
                nc.vector.tensor_reduce(dd[:], tmp[:], axis=mybir.AxisListType.X, op=ALU.add)
                nc.vector.tensor_tensor(out=tmp[:], in0=Sk[k][:], in1=dinvb[:], op=ALU.mult)
                ds_ = cp.tile([128, 1], F32, tag="dsk")
                nc.vector.tensor_reduce(ds_[:], tmp[:], axis=mybir.AxisListType.X, op=ALU.add)
                nrm = cp.tile([128, 1], F32, tag="nrmk")
                nc.vector.tensor_tensor(out=nrm[:], in0=dd[:], in1=ds_[:], op=ALU.mult)
                sn = cp.tile([128, N], F32, tag=f"snk{k}")
                nc.vector.tensor_scalar(out=sn[:], in0=Sk[k][:], scalar1=nrm[:, :1],
                                        scalar2=None, op0=ALU.mult)
                nc.tensor.matmul(at_ps[:], sn[:], Dk[k][:], start=(k == 0), stop=False)
            diagd = cp.tile([N, N], F32)
            nc.vector.tensor_tensor(out=diagd[:], in0=id128[:N, :N], in1=dinvb[:N, :], op=ALU.mult)
            nc.tensor.matmul(at_ps[:], diagd[:], diagd[:], start=False, stop=True)
            at_sb = cp.tile([N, N], BF16)
            nc.vector.tensor_copy(at_sb[:], at_ps[:])
            BDb = cp.tile([128, 128], BF16)
            nc.vector.memset(BDb[:], 0.0)
            for q in range(4):
                nc.sync.dma_start(out=BDb[32 * q:32 * (q + 1), 32 * q:32 * (q + 1)], in_=at_sb[:])

            # ---------------- fused gate weights + beta row ----------------
            mzh = cp.tile([EMB, 128], BF16)
            brow = cp.tile([1, 128], F32)
            for gi, (ow, ob, olb, olg, scale) in enumerate((
                    (OWZ, OBZ, OLBZ, OLZ, -0.5), (OWH, OBH, OLBH, OLH, 1.0))):
                wT_ps = pp1.tile([HID, EMB], F32, tag="p1")
                nc.tensor.transpose(wT_ps[:], blob_sb[0:EMB, ow:ow + HID], id128[:EMB, :EMB])
                wT = cp.tile([HID, EMB], F32, tag=f"wt{gi}")
                nc.vector.tensor_copy(wT[:], wT_ps[:])
                m_ps = pp1.tile([EMB, HID], F32, tag="p1")
                nc.tensor.matmul(m_ps[:], wT[:], blob_sb[0:HID, olg:olg + HID], start=True, stop=True)
                nc.scalar.activation(mzh[:, 64 * gi:64 * (gi + 1)], m_ps[:], AF.Copy, scale=scale)
                bb_ps = pp1.tile([HID, 1], F32, tag="p1")
                nc.tensor.matmul(bb_ps[:], blob_sb[0:HID, olg:olg + HID], bcol(ob), start=True, stop=True)
                bsum = cp.tile([HID, 1], F32, tag=f"bsum{gi}")
                nc.vector.tensor_tensor(out=bsum[:], in0=bb_ps[:], in1=bcol(olb), op=ALU.add)
                bscl = cp.tile([HID, 1], F32, tag=f"bscl{gi}")
                nc.scalar.activation(bscl[:], bsum[:], AF.Copy, scale=scale)
                brt_ps = pp1.tile([1, HID], F32, tag="p1")
                nc.tensor.transpose(brt_ps[:], bscl[:], id128[:HID, :HID])
                nc.vector.tensor_copy(brow[0:1, 64 * gi:64 * (gi + 1)], brt_ps[:])
            brow4 = cp.tile([1, 512], F32)
            for rr in range(4):
                nc.vector.tensor_copy(brow4[0:1, 128 * rr:128 * (rr + 1)], brow[:])
            brows_ps = pp1.tile([128, 512], F32, tag="p1")
            nc.tensor.matmul(brows_ps[:], ones_row[:], brow4[:], start=True, stop=True)
            brows = cp.tile([128, 512], BF16)
            nc.vector.tensor_copy(brows[:], brows_ps[:])

            # ---------------- LOS blend -> pooling matrix Gall --------------
            losf = cp.tile([1, BSH], F32)
            nc.vector.tensor_copy(losf[:], los_sb[:])
            losb_ps = pp1.tile([T, BSH], F32, tag="p1")
            nc.tensor.matmul(losb_ps[:], ones_row[:1, :T], losf[:], start=True, stop=True)
            mask = cp.tile([T, BSH], F32)
            nc.vector.tensor_tensor(out=mask[:], in0=blob_sb[0:T, OIOT:OIOT + 1].to_broadcast([T, BSH]),
                                    in1=losb_ps[:], op=ALU.is_lt)
            ecol = cp.tile([T, 1], F32)
            nc.scalar.activation(ecol[:], blob_sb[0:T, OATT:OATT + 1], AF.Exp)
            # preload the tanh ACT table here (idle window) so phase 2's first
            # tanh doesn't pay the 1.28us table swap on the critical path
            tanh_warm = cp.tile([1, 1], F32)
            nc.scalar.activation(tanh_warm[:], ones_col[0:1, 0:1], AF.Tanh)
            esum_ps = pp1.tile([1, 1], F32, tag="p1")
            nc.tensor.matmul(esum_ps[:], ecol[:], ones_col[:T, :], start=True, stop=True)
            rinv = cp.tile([1, 1], F32)
            nc.vector.reciprocal(rinv[:], esum_ps[:])
            wraw_ps = pp1.tile([1, BSH], F32, tag="p1")
            nc.tensor.matmul(wraw_ps[:], ecol[:], mask[:], start=True, stop=True)
            wrow = cp.tile([1, BSH], F32)
            nc.vector.tensor_scalar(out=wrow[:], in0=wraw_ps[:], scalar1=rinv[:, :1],
                                    scalar2=None, op0=ALU.mult)
            w128_ps = pp1.tile([128, BSH], F32, tag="p1")
            nc.tensor.matmul(w128_ps[:], ones_row[:], wrow[:], start=True, stop=True)
            gtmp = cp.tile([128, BSH], F32)
            nc.vector.tensor_tensor(out=gtmp[:], in0=w128_ps[:], in1=blob_sb[:, OPDIF:OPDIF + BSH],
                                    op=ALU.mult)
            gall = cp.tile([128, BSH], BF16)
            nc.vector.tensor_tensor(out=gall[:], in0=gtmp[:], in1=blob_sb[:, OPDIS:OPDIS + BSH],
                                    op=ALU.add)

            # ---------------- phase 1: one-hots -> X^T -> Q^T ----------------
            i100b = cp.tile([V, 1], I8)
            nc.vector.tensor_copy(i100b[:], blob_sb[0:V, OI100:OI100 + 1])
            o_sb = cp.tile([V, R], BF16)
            qt = cp.tile([128, R], BF16)
            qtc = cp.tile([128, R], BF16)

            def p1_a(k):
                # x values replicated as int8: 400KB instead of 1.6MB of DMA
                xrep = wp.tile([V, 512], I8, tag="xrep")
                t = xbf[512 * k:512 * (k + 1)]
                nc.sync.dma_start(out=xrep[:], in_=bass.AP(t.tensor, t.offset, [[0, V]] + list(t.ap)))
                nc.vector.tensor_tensor(out=o_sb[:, 512 * k:512 * (k + 1)],
                                        in0=i100b[:, :1].to_broadcast([V, 512]),
                                        in1=xrep[:], op=ALU.is_equal)
                xt_ps = pp1.tile([EMB, 512], F32, tag="p1")
                for cc in range(8):
                    c = 8 * k + cc
                    nc.tensor.matmul(xt_ps[:, 64 * cc:64 * (cc + 1)],
                                     tp3_sb[:, EMB * c:EMB * (c + 1)],
                                     o_sb[:, 64 * c:64 * (c + 1)], start=True, stop=True)
                xt = wp.tile([EMB, 512], BF16, tag="xt")
                nc.scalar.activation(xt[:], xt_ps[:], AF.Copy)
                return xt

            # qt is stored b-major (col = b*64 + c) so phase-2 transposes can
            # read plain [128, 128] slices; the copy scatters via a strided AP
            qt3v = qt[:].rearrange("p (b c) -> p c b", c=C)

            def p1_b(k, xt):
                q_ps = pp1.tile([128, 512], F32, tag="p1")
                nc.tensor.matmul(q_ps[:], mzh[:], xt[:], start=True, stop=True)
                # the (c,b)->(b,c) scatter costs ~2.5us per 512 cols on any
                # engine; spread the 8 of them across ACT / GPSIMD / DVE
                dst = qt3v[:, 8 * k:8 * (k + 1), :]
                srcv = q_ps[:].rearrange("p (c b) -> p c b", c=8)
                if k in (0, 3, 6):
                    nc.scalar.activation(dst, srcv, AF.Copy)
                elif k in (1, 2, 4, 7):
                    nc.scalar.activation(qtc[:, 512 * k:512 * (k + 1)], q_ps[:], AF.Copy)
                    nc.gpsimd.tensor_copy(
                        dst, qtc[:, 512 * k:512 * (k + 1)].rearrange("p (c b) -> p c b", c=8))
                else:
                    nc.vector.tensor_copy(dst, srcv)

            # phase 1 only needs blob/tp3/xbf - let it win scheduler ties
            # over the adjacency/gate prep chain emitted above
            xts = {}
            with tc.high_priority():
                for k in range(NBATCH + 1):
                    if k < NBATCH:
                        xts[k] = p1_a(k)
                    if k >= 1:
                        p1_b(k - 1, xts.pop(k - 1))

            # ---------------- phase 2: per-chunk transpose/adjacency/gates ---
            pool_ps = ppA.tile([HID, BSH], F32, tag="pA")
            NQUAD = NCHUNK // 4

            def p2_a(p):
                # four chunk transposes into one psum tile
                tr_ps = ppT.tile([128, 512], BF16)
                for h in range(4):
                    nc.tensor.transpose(tr_ps[:, 128 * h:128 * (h + 1)],
                                        qt[:, 512 * p + 128 * h:512 * p + 128 * (h + 1)],
                                        idb[:])
                return tr_ps

            def p2_b(p, tr_ps):
                qr = wp.tile([128, 512], BF16, tag="qr")
                nc.vector.tensor_copy(qr[:], tr_ps[:])
                s_ps = ppS2.tile([128, 512], F32)
                nc.tensor.matmul(s_ps[:], BDb[:], qr[:], start=True, stop=True)
                return s_ps

            def p2_c(p, s_ps):
                sb = wp.tile([128, 512], BF16, tag="sb")
                nc.vector.tensor_tensor(out=sb[:], in0=s_ps[:], in1=brows[:], op=ALU.add)
                u = wp.tile([128, 512], BF16, tag="u")
                nc.scalar.activation(u[:], sb[:], AF.Tanh)
                u4 = u[:].rearrange("q (k g o) -> q k g o", k=4, g=2)
                w1 = wp.tile([128, 4, HID], BF16, tag="w1")
                nc.vector.tensor_tensor(out=w1[:], in0=u4[:, :, 0, :], in1=u4[:, :, 1, :],
                                        op=ALU.mult)
                w2 = wp.tile([128, 4, HID], BF16, tag="w2")
                nc.vector.tensor_tensor(out=w2[:], in0=w1[:], in1=u4[:, :, 1, :], op=ALU.add)
                for h in range(4):
                    j = 4 * p + h
                    nc.tensor.matmul(pool_ps[:, 2 * j:2 * (j + 1)], w2[:, h, :],
                                     gall[:, 2 * j:2 * (j + 1)], start=True, stop=True)

            st_a, st_b = {}, {}
            for p in range(NQUAD + 2):
                if p < NQUAD:
                    st_a[p] = p2_a(p)
                if 1 <= p <= NQUAD:
                    st_b[p - 1] = p2_b(p - 1, st_a.pop(p - 1))
                if 2 <= p:
                    p2_c(p - 2, st_b.pop(p - 2))

            # ---------------- classifier ----------------
            pt = cp.tile([HID, BSH], F32)
            nc.vector.tensor_copy(pt[:], pool_ps[:])
            u1_ps = pp1.tile([2 * HID, BSH], F32, tag="p1")
            nc.tensor.matmul(u1_ps[:], blob_sb[0:HID, OWC1:OWC1 + 2 * HID], pt[:], start=True, stop=True)
            v = cp.tile([2 * HID, BSH], F32)
            nc.scalar.activation(v[:], u1_ps[:], AF.Relu, bias=bcol(OBC1, 128), scale=1.0 / 64.0)
            y_ps2 = pp1.tile([1, BSH], F32, tag="p1")
            nc.tensor.matmul(y_ps2[:], blob_sb[0:128, OWC2:OWC2 + 1], v[:], start=True, stop=True)
            yrow = cp.tile([1, BSH], F32)
            nc.scalar.activation(yrow[:], y_ps2[:], AF.Identity, bias=bcol(OBC2, 1))
            nc.sync.dma_start(out=out[:], in_=yrow[:])

    nc.finalize()
    return nc


def _stage(inputs):
    """Host-side staging: shard + pack.  Pure layout work, no model math."""
    x_batch = np.asarray(inputs["x_batch"]).astype(np.int32)
    los = np.asarray(inputs["LOS_batch"]).astype(np.int32)
    edge = np.asarray(inputs["template_edge_index"]).astype(np.int32)
    emb = np.asarray(inputs["emb_table"], dtype=np.float32)

    tp = np.zeros((C * V, 64), np.float32)
    tp[:, :EMB] = emb.reshape(C * V, EMB)

    blob = np.zeros((128, BLOBF), np.float32)
    blob[0:EMB, OWZ:OWZ + HID] = inputs["W_z"]
    blob[0:EMB, OWH:OWH + HID] = inputs["W_h"]
    blob[0:HID, OLZ:OLZ + HID] = np.asarray(inputs["L_z"])[:HID]
    blob[0:HID, OLH:OLH + HID] = np.asarray(inputs["L_h"])[:HID]
    blob[0:HID, OWC1:OWC1 + 2 * HID] = inputs["Wc1"]
    blob[0:HID, OBZ] = inputs["b_z"]
    blob[0:HID, OBH] = inputs["b_h"]
    blob[0:HID, OLBZ] = inputs["lb_z"]
    blob[0:HID, OLBH] = inputs["lb_h"]
    blob[0:2 * HID, OWC2] = np.asarray(inputs["Wc2"])[:, 0]
    blob[0:2 * HID, OBC1] = inputs["bc1"]
    blob[0, OBC2] = np.asarray(inputs["bc2"])[0]
    blob[0:T, OATT] = inputs["att"]
    blob[:, OID:OID + 128] = np.eye(128, dtype=np.float32)
    blob[0, OION:OION + N] = np.arange(N, dtype=np.float32)
    blob[0:T, OIOT] = np.arange(T, dtype=np.float32)
    blob[0:V, OI100] = np.arange(V, dtype=np.float32)
    # pooling/blend selection patterns: col b, chunk j=b//2, q=b%2
    # ad rows 64q..64q+32, dis rows 64q+32..64q+64
    p = np.arange(128)[:, None]
    b = np.arange(BSH)[None, :]
    p_ad = (p // 32 == 2 * (b % 2)).astype(np.float32)
    p_dis = (p // 32 == 2 * (b % 2) + 1).astype(np.float32)
    blob[:, OPDIF:OPDIF + BSH] = p_ad - p_dis
    blob[:, OPDIS:OPDIS + BSH] = p_dis

    col_off = (np.arange(C, dtype=np.int32) * V)[None, :]
    in_maps = []
    for i in range(NCORES):
        xs = x_batch[i * BSH:(i + 1) * BSH]            # [64, 64]
        flat = (xs + col_off).astype(np.int16).ravel()  # row r = b*64+c
        wrapped = np.tile(flat.reshape(R // 16, 16).T, (8, 1)).copy()  # [128, R//16]
        in_maps.append({
            "tp": tp,
            "gidx": wrapped,
            "edge": edge,
            "los": los[i * BSH:(i + 1) * BSH].reshape(1, BSH).copy(),
            "blob": blob,
        })
    return in_maps


def _stage_v2(inputs):
    """Host staging for the PE-one-hot kernel: transposed bf16 table +
    x values as f32 in (c-major, b-minor) order."""
    x_batch = np.asarray(inputs["x_batch"]).astype(np.int32)
    emb = np.asarray(inputs["emb_table"], dtype=np.float32)
    base = _stage(inputs)
    # [v, (c, e)] layout, bf16
    import ml_dtypes
    tp3 = np.ascontiguousarray(
        emb.transpose(1, 0, 2).reshape(V, C * EMB)).astype(ml_dtypes.bfloat16)
    in_maps = []
    for i in range(NCORES):
        xs = x_batch[i * BSH:(i + 1) * BSH]                 # [64 b, 64 c]
        xbf = np.ascontiguousarray(xs.T).reshape(-1).astype(np.int8)  # c-major
        m = {k: base[i][k] for k in ("edge", "los", "blob")}
        m["tp3"] = tp3
        m["xbf"] = xbf
        in_maps.append(m)
    return in_maps


def kernel(**inputs) -> np.ndarray:
    global LAST_EXEC_NS
    ver = os.environ.get("BASSKERNEL_VER", "2")
    if ver not in _CACHE:
        _CACHE[ver] = _build_nc_v2() if ver == "2" else _build_nc()
    nc = _CACHE[ver]
    in_maps = _stage_v2(inputs) if ver == "2" else _stage(inputs)
    trace = bool(int(os.environ.get("BASSKERNEL_TRACE", "0")))
    kw = {}
    if trace:
        _install_ntff_hook()
        kw["trace"] = True
        tmpdir = os.environ.get("BASSKERNEL_TMPDIR")
        if tmpdir:
            kw["tmpdir"] = tmpdir
    res = run_bass_kernel_spmd(nc, in_maps, core_ids=list(range(NCORES)), **kw)
    LAST_EXEC_NS = getattr(res, "exec_time_ns", None)
    out = np.empty((B, 1), np.float32)
    for i in range(NCORES):
        out[i * BSH:(i + 1) * BSH, 0] = np.asarray(res.results[i]["out"]).reshape(BSH)
    return out


# revision 37
# speedup vs baseline: 1.1587x; 1.1587x over previous
"""Trainium2 Bass kernel for the A3TGCN-2-points model (8 NeuronCores, data-parallel).

Math (verified vs a line-by-line port of the reference at 3.5e-8):
  - The reference passes H=None each period, so H0 = 0: the reset gate R
    vanishes and only the first HID rows of L_z / L_h matter.
  - x_temporal takes two values per sample (admission cols before t < LOS,
    discharge after), so the 37-step attention scan collapses to
        H = w * cell(ad) + (1 - w) * cell(dis),  w = cumsum(softmax(att))[LOS]
    cell(X) = (1 - sigmoid(A X Wz Lz1 + beta_z)) * tanh(A X Wh Lh1 + beta_h)
  - 1 - sigmoid(s) = (1 - tanh(s/2)) / 2, so ONE tanh evaluates both gates
    (the z columns of the fused weights are scaled by -1/2).

Sharding: batch 512 -> 64 samples per core; all weights replicated; the
[64, 1] logits per core are concatenated on the host.

Device mapping, default version 2 (BASSKERNEL_VER=1 selects the dma_gather
variant):
  - embedding lookup via PE one-hot matmuls from a host-transposed [v,(c,e)]
    table (one [100,32]x[100,64] matmul per column) -> X^T, e on partitions
  - fused gate weights Mzh = [-Wz@Lz1/2 | Wh@Lh1] applied as one matmul per
    512 columns -> Q^T, then a (c,b)->(b,c) free-axis reorder (split across
    ACT/GPSIMD/DVE - it costs ~2.5us/512 cols on any single engine)
  - per 128-row chunk: PE transpose, adjacency I4 (x) A^T matmul (gcn_norm
    built on device from edge_index via one-hot matmuls), bias add, tanh,
    gate combine, and pooling+LOS-blend fused into a [128,64]x[128,2] matmul
  - classifier MLP on the pooled [64, 64] tile, 64 f32 out per core.
"""

import os
import sys

import numpy as np

sys.path.insert(0, "/opt/trn_rl_repo")

import concourse.bacc as bacc
import concourse.bass as bass
import concourse.mybir as mybir
import concourse.tile as tile
from concourse.bass_utils import run_bass_kernel_spmd

F32 = mybir.dt.float32
I32 = mybir.dt.int32
I16 = mybir.dt.int16
I8 = mybir.dt.int8
BF16 = mybir.dt.bfloat16
AF = mybir.ActivationFunctionType
ALU = mybir.AluOpType

B, C, N, V, EMB, HID, E, T = 512, 64, 32, 100, 32, 64, 256, 37
NCORES = 8
BSH = B // NCORES            # samples per core
R = BSH * C                  # gathered rows per core (4096)
NCHUNK = R // 128            # 32 row-chunks of 128
NBATCH = NCHUNK // 4         # 8 batches of 4 chunks ([*, 512] tiles)

# blob column layout (weights packed into one [128, 392] f32 DMA)
OWZ, OWH, OLZ, OLH, OWC1 = 0, 64, 128, 192, 256
OBZ, OBH, OLBZ, OLBH, OWC2, OBC1, OBC2, OATT = 384, 385, 386, 387, 388, 389, 390, 391
OID, OION, OIOT = 392, 520, 552
OI100, OPDIF, OPDIS = 553, 554, 618
BLOBF = 682


def _install_ntff_hook():
    """The agent image's antenv lacks axon_hooks; synthesize it so trace=True
    can drive NTFF profiling via ctypes on libaxon_pjrt.so (mirrors the
    boot-side hook in trn_boot.py)."""
    import contextlib
    import ctypes
    import types

    if "antenv.axon_hooks" in sys.modules:
        return
    so_path = "/opt/axon/libaxon_pjrt.so"
    mod = types.ModuleType("antenv.axon_hooks")
    state = {"hook": None}

    def set_axon_ntff_profile_hook(h):
        state["hook"] = h

    def get_axon_ntff_profile_hook():
        return state["hook"]

    mod.set_axon_ntff_profile_hook = set_axon_ntff_profile_hook
    mod.get_axon_ntff_profile_hook = get_axon_ntff_profile_hook
    sys.modules["antenv.axon_hooks"] = mod
    try:
        import antenv
        antenv.axon_hooks = mod
    except ImportError:
        pass

    if not os.path.exists(so_path):
        return
    lib = ctypes.CDLL(so_path)
    if not hasattr(lib, "axon_start_nrt_profile"):
        return
    lib.axon_start_nrt_profile.argtypes = [ctypes.POINTER(ctypes.c_int64), ctypes.c_size_t]
    lib.axon_start_nrt_profile.restype = ctypes.c_int64
    lib.axon_stop_nrt_profile.argtypes = [ctypes.c_char_p]
    lib.axon_stop_nrt_profile.restype = ctypes.c_int64

    @contextlib.contextmanager
    def _hook(output_dir, device_ids):
        import jax
        jax.devices()
        if device_ids:
            ids = (ctypes.c_int64 * len(device_ids))(*device_ids)
            rc = lib.axon_start_nrt_profile(ids, len(device_ids))
        else:
            rc = lib.axon_start_nrt_profile(None, 0)
        if rc != 0:
            raise RuntimeError(f"axon_start_nrt_profile rc={rc}")
        try:
            yield
        finally:
            n = lib.axon_stop_nrt_profile(str(output_dir).encode())
            print(f"profile: {n} file(s) written to {output_dir}", file=sys.stderr)

    set_axon_ntff_profile_hook(_hook)


_CACHE = {}
LAST_EXEC_NS = None


def _build_nc():
    nc = bacc.Bacc("TRN2")

    tp = nc.declare_dram_parameter("tp", [C * V, 64], F32, isOutput=False)
    gidx = nc.declare_dram_parameter("gidx", [128, R // 16], I16, isOutput=False)
    edge = nc.declare_dram_parameter("edge", [2, E], I32, isOutput=False)
    los = nc.declare_dram_parameter("los", [1, BSH], I32, isOutput=False)
    blob = nc.declare_dram_parameter("blob", [128, BLOBF], F32, isOutput=False)
    out = nc.declare_dram_parameter("out", [1, BSH], F32, isOutput=True)

    with tile.TileContext(nc) as tc:
        with (
            tc.tile_pool(name="const", bufs=1) as cp,
            tc.tile_pool(name="work", bufs=3) as wp,
            tc.tile_pool(name="ppY", bufs=3, space="PSUM") as ppY,
            tc.tile_pool(name="ppS", bufs=2, space="PSUM") as ppS,
            tc.tile_pool(name="ppA", bufs=1, space="PSUM") as ppA,
        ):
            # ---------------- input DMAs ----------------
            gsb = cp.tile([128, R // 16], I16)
            nc.sync.dma_start(out=gsb[:], in_=gidx[:])
            # dummy 16-row dma_gather: forces the Q7 mlp library load to
            # overlap the input DMAs instead of delaying the first real gather
            warm_idx = cp.tile([128, 1], I16)
            nc.vector.memset(warm_idx[:], 0)
            warm_out = cp.tile([128, 1, 64], F32)
            nc.gpsimd.dma_gather(
                out_ap=warm_out[:], in_ap=tp[:], idxs_ap=warm_idx[:],
                num_idxs=16, num_idxs_reg=16, elem_size=64)
            blob_sb = cp.tile([128, BLOBF], F32)
            nc.sync.dma_start(out=blob_sb[:], in_=blob[:])
            esrc = cp.tile([128, 2], I32)
            nc.sync.dma_start(out=esrc[:], in_=edge[0].rearrange("(k p) -> p k", p=128))
            edst = cp.tile([128, 2], I32)
            nc.sync.dma_start(out=edst[:], in_=edge[1].rearrange("(k p) -> p k", p=128))
            los_sb = cp.tile([1, BSH], I32)
            nc.sync.dma_start(out=los_sb[:], in_=los[:])

            def bcol(off, rows=64):
                return blob_sb[0:rows, off:off + 1]

            # ---------------- embedding gather ----------------
            # the SWDGE descriptor ring tops out between 1k and 2k entries per
            # shot; 4 gathers of 1024 rows, interleaved with the batches that
            # consume them (emitted in the main loop below)
            xg = cp.tile([128, NCHUNK, 64], F32)
            GCH = 1024

            def issue_gather(c0, c1):
                # gathers rows for chunks [c0, c1)
                nc.gpsimd.dma_gather(
                    out_ap=xg[:, c0:c1, :],
                    in_ap=tp[:],
                    idxs_ap=gsb[:, 8 * c0:8 * c1],
                    num_idxs=128 * (c1 - c0),
                    num_idxs_reg=128 * (c1 - c0),
                    elem_size=64,
                )

            # ---------------- constants ----------------
            id128 = blob_sb[:, OID:OID + 128]
            ones_col = cp.tile([128, 1], F32)
            nc.vector.memset(ones_col[:], 1.0)
            ones_row = cp.tile([1, 128], F32)
            nc.vector.memset(ones_row[:], 1.0)
            iota_nf = cp.tile([128, N], F32)
            _src = blob[0, OION:OION + N]
            nc.sync.dma_start(out=iota_nf[:], in_=bass.AP(_src.tensor, _src.offset, [[0, 128]] + list(_src.ap)))

            # ---------------- adjacency build: BD = I4 (x) A^T ----------------
            srcf = cp.tile([128, 2], F32)
            nc.vector.tensor_copy(srcf[:], esrc[:])
            dstf = cp.tile([128, 2], F32)
            nc.vector.tensor_copy(dstf[:], edst[:])

            Dk, Sk = [], []
            for k in range(2):
                d = cp.tile([128, N], F32, tag=f"dk{k}")
                nc.vector.tensor_tensor(
                    out=d[:], in0=dstf[:, k:k + 1].to_broadcast([128, N]),
                    in1=iota_nf[:], op=ALU.is_equal)
                s = cp.tile([128, N], F32, tag=f"sk{k}")
                nc.vector.tensor_tensor(
                    out=s[:], in0=srcf[:, k:k + 1].to_broadcast([128, N]),
                    in1=iota_nf[:], op=ALU.is_equal)
                Dk.append(d)
                Sk.append(s)

            deg_ps = ppS.tile([1, N], F32, tag="s_ps")
            nc.tensor.matmul(deg_ps[:], ones_col[:], Dk[0][:], start=True, stop=False)
            nc.tensor.matmul(deg_ps[:], ones_col[:], Dk[1][:], start=False, stop=True)
            degp1 = cp.tile([1, N], F32)
            nc.scalar.activation(degp1[:], deg_ps[:], AF.Identity, bias=1.0)
            rec = cp.tile([1, N], F32)
            nc.vector.reciprocal(rec[:], degp1[:])
            dinv_row = cp.tile([1, N], F32)
            nc.scalar.activation(dinv_row[:], rec[:], AF.Sqrt)

            dinvb_ps = ppS.tile([128, N], F32, tag="s_ps")
            nc.tensor.matmul(dinvb_ps[:], ones_row[:], dinv_row[:], start=True, stop=True)
            dinvb = cp.tile([128, N], F32)
            nc.vector.tensor_copy(dinvb[:], dinvb_ps[:])

            at_ps = ppA.tile([N, N], F32)
            for k in range(2):
                tmp = cp.tile([128, N], F32, tag="degtmp")
                nc.vector.tensor_tensor(out=tmp[:], in0=Dk[k][:], in1=dinvb[:], op=ALU.mult)
                dd = cp.tile([128, 1], F32, tag="ddk")
                nc.vector.tensor_reduce(dd[:], tmp[:], axis=mybir.AxisListType.X, op=ALU.add)
                nc.vector.tensor_tensor(out=tmp[:], in0=Sk[k][:], in1=dinvb[:], op=ALU.mult)
                ds_ = cp.tile([128, 1], F32, tag="dsk")
                nc.vector.tensor_reduce(ds_[:], tmp[:], axis=mybir.AxisListType.X, op=ALU.add)
                nrm = cp.tile([128, 1], F32, tag="nrmk")
                nc.vector.tensor_tensor(out=nrm[:], in0=dd[:], in1=ds_[:], op=ALU.mult)
                sn = cp.tile([128, N], F32, tag=f"snk{k}")
                nc.vector.tensor_scalar(out=sn[:], in0=Sk[k][:], scalar1=nrm[:, :1],
                                        scalar2=None, op0=ALU.mult)
                nc.tensor.matmul(at_ps[:], sn[:], Dk[k][:], start=(k == 0), stop=False)
            diagd = cp.tile([N, N], F32)
            nc.vector.tensor_tensor(out=diagd[:], in0=id128[:N, :N], in1=dinvb[:N, :],
                                    op=ALU.mult)
            nc.tensor.matmul(at_ps[:], diagd[:], diagd[:], start=False, stop=True)

            # engines are lane-locked (no partition shifts), so place the four
            # diagonal blocks with SBUF->SBUF DMAs
            at_sb = cp.tile([N, N], F32)
            nc.vector.tensor_copy(at_sb[:], at_ps[:])
            BD = cp.tile([128, 128], F32)
            nc.vector.memset(BD[:], 0.0)
            for q in range(4):
                nc.sync.dma_start(out=BD[32 * q:32 * (q + 1), 32 * q:32 * (q + 1)],
                                  in_=at_sb[:])

            # ---------------- fused gate weights Mzh = [-Mz/2 | Mh] ----------------
            mzh = cp.tile([EMB, 128], F32)
            betas = []
            for gi, (ow, ob, olb, olg, scale) in enumerate((
                    (OWZ, OBZ, OLBZ, OLZ, -0.5), (OWH, OBH, OLBH, OLH, 1.0))):
                wT_ps = ppS.tile([HID, EMB], F32, tag="s_ps")
                nc.tensor.transpose(wT_ps[:], blob_sb[0:EMB, ow:ow + HID], id128[:EMB, :EMB])
                wT = cp.tile([HID, EMB], F32, tag=f"wt{gi}")
                nc.vector.tensor_copy(wT[:], wT_ps[:])
                m_ps = ppS.tile([EMB, HID], F32, tag="s_ps")
                nc.tensor.matmul(m_ps[:], wT[:], blob_sb[0:HID, olg:olg + HID],
                                 start=True, stop=True)
                nc.scalar.activation(mzh[:, 64 * gi:64 * (gi + 1)], m_ps[:], AF.Copy,
                                     scale=scale)
                # beta_g = Lg1^T b_g + lb_g  (as a column), scaled like Mz/Mh
                bb_ps = ppS.tile([HID, 1], F32, tag="s_ps")
                nc.tensor.matmul(bb_ps[:], blob_sb[0:HID, olg:olg + HID], bcol(ob),
                                 start=True, stop=True)
                bsum = cp.tile([HID, 1], F32, tag=f"bsum{gi}")
                nc.vector.tensor_tensor(out=bsum[:], in0=bb_ps[:], in1=bcol(olb), op=ALU.add)
                bcolg = cp.tile([HID, 1], F32, tag=f"beta{gi}")
                nc.scalar.activation(bcolg[:], bsum[:], AF.Copy, scale=scale)
                betas.append(bcolg)

            # ---------------- LOS blend weights ----------------
            losf = cp.tile([1, BSH], F32)
            nc.vector.tensor_copy(losf[:], los_sb[:])
            losb_ps = ppS.tile([T, BSH], F32, tag="s_ps")
            nc.tensor.matmul(losb_ps[:], ones_row[:1, :T], losf[:], start=True, stop=True)
            mask = cp.tile([T, BSH], F32)
            nc.vector.tensor_tensor(out=mask[:], in0=blob_sb[0:T, OIOT:OIOT + 1].to_broadcast([T, BSH]),
                                    in1=losb_ps[:], op=ALU.is_lt)
            ecol = cp.tile([T, 1], F32)
            nc.scalar.activation(ecol[:], blob_sb[0:T, OATT:OATT + 1], AF.Exp)
            esum_ps = ppS.tile([1, 1], F32, tag="s_ps")
            nc.tensor.matmul(esum_ps[:], ecol[:], ones_col[:T, :], start=True, stop=True)
            rinv = cp.tile([1, 1], F32)
            nc.vector.reciprocal(rinv[:], esum_ps[:])
            wraw_ps = ppS.tile([1, BSH], F32, tag="s_ps")
            nc.tensor.matmul(wraw_ps[:], ecol[:], mask[:], start=True, stop=True)
            wrow = cp.tile([1, BSH], F32)
            nc.vector.tensor_scalar(out=wrow[:], in0=wraw_ps[:], scalar1=rinv[:, :1],
                                    scalar2=None, op0=ALU.mult)
            wb_ps = ppS.tile([HID, BSH], F32, tag="s_ps")
            nc.tensor.matmul(wb_ps[:], ones_row[:1, :HID], wrow[:], start=True, stop=True)
            wb = cp.tile([HID, BSH], F32)
            nc.vector.tensor_copy(wb[:], wb_ps[:])

            # ---------------- main loop ----------------
            sums = cp.tile([HID, 2 * BSH], F32)
            gather_plan = {0: (0, 8), 2: (8, 16), 4: (16, 24), 6: (24, 28), 7: (28, 32)}
            for jb in range(NBATCH):
                if jb in gather_plan:
                    issue_gather(*gather_plan[jb])
                y_ps = ppY.tile([EMB, 512], F32)
                for jj in range(4):
                    j = 4 * jb + jj
                    nc.tensor.matmul(y_ps[:, 128 * jj:128 * (jj + 1)],
                                     xg[:, j, 0:EMB], BD[:], start=True, stop=True)
                ysb = wp.tile([EMB, 512], F32)
                nc.vector.tensor_copy(ysb[:], y_ps[:])
                # z- and h-gate pre-activations side by side on the SAME
                # partitions (engines cannot shift lanes)
                s_ps = ppS.tile([HID, 1024], F32, tag="s_ps")
                nc.tensor.matmul(s_ps[:, 0:512], mzh[:, 0:64], ysb[:],
                                 start=True, stop=True)
                nc.tensor.matmul(s_ps[:, 512:1024], mzh[:, 64:128], ysb[:],
                                 start=True, stop=True)
                u = wp.tile([HID, 1024], BF16)
                nc.scalar.activation(u[:, 0:512], s_ps[:, 0:512], AF.Tanh,
                                     bias=betas[0][:, :1])
                nc.scalar.activation(u[:, 512:1024], s_ps[:, 512:1024], AF.Tanh,
                                     bias=betas[1][:, :1])
                w1 = wp.tile([HID, 512], BF16)
                nc.vector.tensor_tensor(out=w1[:], in0=u[:, 0:512], in1=u[:, 512:1024],
                                        op=ALU.mult)
                w2 = wp.tile([HID, 512], BF16)
                nc.vector.tensor_tensor(out=w2[:], in0=w1[:], in1=u[:, 512:1024],
                                        op=ALU.add)
                nc.vector.tensor_reduce(
                    sums[:, 16 * jb:16 * (jb + 1)],
                    w2[:].rearrange("p (g n) -> p g n", n=N),
                    axis=mybir.AxisListType.X, op=ALU.add)

            # ---------------- blend + pool + classifier ----------------
            s3 = sums[:].rearrange("p (s k) -> p s k", k=2)
            t1 = cp.tile([HID, BSH], F32)
            nc.vector.tensor_tensor(out=t1[:], in0=s3[:, :, 0], in1=s3[:, :, 1],
                                    op=ALU.subtract)
            t2 = cp.tile([HID, BSH], F32)
            nc.vector.tensor_tensor(out=t2[:], in0=t1[:], in1=wb[:], op=ALU.mult)
            pt = cp.tile([HID, BSH], F32)
            nc.vector.tensor_tensor(out=pt[:], in0=t2[:], in1=s3[:, :, 1], op=ALU.add)

            u1_ps = ppS.tile([2 * HID, BSH], F32, tag="s_ps")
            nc.tensor.matmul(u1_ps[:], blob_sb[0:HID, OWC1:OWC1 + 2 * HID], pt[:],
                             start=True, stop=True)
            v = cp.tile([2 * HID, BSH], F32)
            nc.scalar.activation(v[:], u1_ps[:], AF.Relu, bias=bcol(OBC1, 128),
                                 scale=1.0 / 64.0)
            y_ps2 = ppS.tile([1, BSH], F32, tag="s_ps")
            nc.tensor.matmul(y_ps2[:], blob_sb[0:128, OWC2:OWC2 + 1], v[:],
                             start=True, stop=True)
            yrow = cp.tile([1, BSH], F32)
            nc.scalar.activation(yrow[:], y_ps2[:], AF.Identity, bias=bcol(OBC2, 1))
            nc.sync.dma_start(out=out[:], in_=yrow[:])

    nc.finalize()
    return nc



def _build_nc_v2():
    """PE-one-hot variant: no GPSIMD at all (no library load, no descriptor
    prep).  Embedding lookup = per-column one-hot matmuls from a transposed
    bf16 table; adjacency applied on transposed row-chunks; pooling and the
    LOS blend fused into a per-chunk matmul."""
    nc = bacc.Bacc("TRN2")

    tp3 = nc.declare_dram_parameter("tp3", [V, C * EMB], BF16, isOutput=False)
    xbf = nc.declare_dram_parameter("xbf", [R], I8, isOutput=False)
    edge = nc.declare_dram_parameter("edge", [2, E], I32, isOutput=False)
    los = nc.declare_dram_parameter("los", [1, BSH], I32, isOutput=False)
    blob = nc.declare_dram_parameter("blob", [128, BLOBF], F32, isOutput=False)
    out = nc.declare_dram_parameter("out", [1, BSH], F32, isOutput=True)

    with tile.TileContext(nc) as tc:
        with (
            tc.tile_pool(name="const", bufs=1) as cp,
            tc.tile_pool(name="work", bufs=3) as wp,
            tc.tile_pool(name="pp1", bufs=3, space="PSUM") as pp1,
            tc.tile_pool(name="ppT", bufs=2, space="PSUM") as ppT,
            tc.tile_pool(name="ppS2", bufs=2, space="PSUM") as ppS2,
            tc.tile_pool(name="ppA", bufs=1, space="PSUM") as ppA,
        ):
            # ---------------- input DMAs ----------------
            blob_sb = cp.tile([128, BLOBF], F32)
            nc.sync.dma_start(out=blob_sb[:], in_=blob[:])
            tp3_sb = cp.tile([V, C * EMB], BF16)
            nc.sync.dma_start(out=tp3_sb[:], in_=tp3[:])
            esrc = cp.tile([128, 2], I32)
            nc.sync.dma_start(out=esrc[:], in_=edge[0].rearrange("(k p) -> p k", p=128))
            edst = cp.tile([128, 2], I32)
            nc.sync.dma_start(out=edst[:], in_=edge[1].rearrange("(k p) -> p k", p=128))
            los_sb = cp.tile([1, BSH], I32)
            nc.sync.dma_start(out=los_sb[:], in_=los[:])

            def bcol(off, rows=64):
                return blob_sb[0:rows, off:off + 1]

            id128 = blob_sb[:, OID:OID + 128]
            ones_col = cp.tile([128, 1], F32)
            nc.vector.memset(ones_col[:], 1.0)
            ones_row = cp.tile([1, 128], F32)
            nc.vector.memset(ones_row[:], 1.0)
            iota_nf = cp.tile([128, N], F32)
            _src = blob[0, OION:OION + N]
            nc.sync.dma_start(out=iota_nf[:], in_=bass.AP(_src.tensor, _src.offset, [[0, 128]] + list(_src.ap)))
            idb = cp.tile([128, 128], BF16)
            nc.scalar.activation(idb[:], id128, AF.Copy)

            # ---------------- adjacency: BD = I4 (x) A^T  (f32 + bf16) -----
            srcf = cp.tile([128, 2], F32)
            nc.vector.tensor_copy(srcf[:], esrc[:])
            dstf = cp.tile([128, 2], F32)
            nc.vector.tensor_copy(dstf[:], edst[:])
            Dk, Sk = [], []
            for k in range(2):
                d = cp.tile([128, N], F32, tag=f"dk{k}")
                nc.vector.tensor_tensor(out=d[:], in0=dstf[:, k:k + 1].to_broadcast([128, N]),
                                        in1=iota_nf[:], op=ALU.is_equal)
                s = cp.tile([128, N], F32, tag=f"sk{k}")
                nc.vector.tensor_tensor(out=s[:], in0=srcf[:, k:k + 1].to_broadcast([128, N]),
                                        in1=iota_nf[:], op=ALU.is_equal)
                Dk.append(d)
                Sk.append(s)
            deg_ps = pp1.tile([1, N], F32, tag="p1")
            nc.tensor.matmul(deg_ps[:], ones_col[:], Dk[0][:], start=True, stop=False)
            nc.tensor.matmul(deg_ps[:], ones_col[:], Dk[1][:], start=False, stop=True)
            degp1 = cp.tile([1, N], F32)
            nc.scalar.activation(degp1[:], deg_ps[:], AF.Identity, bias=1.0)
            rec = cp.tile([1, N], F32)
            nc.vector.reciprocal(rec[:], degp1[:])
            dinv_row = cp.tile([1, N], F32)
            nc.scalar.activation(dinv_row[:], rec[:], AF.Sqrt)
            dinvb_ps = pp1.tile([128, N], F32, tag="p1")
            nc.tensor.matmul(dinvb_ps[:], ones_row[:], dinv_row[:], start=True, stop=True)
            dinvb = cp.tile([128, N], F32)
            nc.vector.tensor_copy(dinvb[:], dinvb_ps[:])
            at_ps = ppA.tile([N, N], F32, tag="pA")
            for k in range(2):
                tmp = cp.tile([128, N], F32, tag="degtmp")
                nc.vector.tensor_tensor(out=tmp[:], in0=Dk[k][:], in1=dinvb[:], op=ALU.mult)
                dd = cp.tile([128, 1], F32, tag="ddk")
                nc.vector.tensor_reduce(dd[:], tmp[:], axis=mybir.AxisListType.X, op=ALU.add)
                nc.vector.tensor_tensor(out=tmp[:], in0=Sk[k][:], in1=dinvb[:], op=ALU.mult)
                ds_ = cp.tile([128, 1], F32, tag="dsk")
                nc.vector.tensor_reduce(ds_[:], tmp[:], axis=mybir.AxisListType.X, op=ALU.add)
                nrm = cp.tile([128, 1], F32, tag="nrmk")
                nc.vector.tensor_tensor(out=nrm[:], in0=dd[:], in1=ds_[:], op=ALU.mult)
                sn = cp.tile([128, N], F32, tag=f"snk{k}")
                nc.vector.tensor_scalar(out=sn[:], in0=Sk[k][:], scalar1=nrm[:, :1],
                                        scalar2=None, op0=ALU.mult)
                nc.tensor.matmul(at_ps[:], sn[:], Dk[k][:], start=(k == 0), stop=False)
            diagd = cp.tile([N, N], F32)
            nc.vector.tensor_tensor(out=diagd[:], in0=id128[:N, :N], in1=dinvb[:N, :], op=ALU.mult)
            nc.tensor.matmul(at_ps[:], diagd[:], diagd[:], start=False, stop=True)
            at_sb = cp.tile([N, N], BF16)
            nc.vector.tensor_copy(at_sb[:], at_ps[:])
            BDb = cp.tile([128, 128], BF16)
            nc.vector.memset(BDb[:], 0.0)
            for q in range(4):
                nc.sync.dma_start(out=BDb[32 * q:32 * (q + 1), 32 * q:32 * (q + 1)], in_=at_sb[:])

            # ---------------- fused gate weights + beta row ----------------
            mzh = cp.tile([EMB, 128], BF16)
            brow = cp.tile([1, 128], F32)
            for gi, (ow, ob, olb, olg, scale) in enumerate((
                    (OWZ, OBZ, OLBZ, OLZ, -0.5), (OWH, OBH, OLBH, OLH, 1.0))):
                wT_ps = pp1.tile([HID, EMB], F32, tag="p1")
                nc.tensor.transpose(wT_ps[:], blob_sb[0:EMB, ow:ow + HID], id128[:EMB, :EMB])
                wT = cp.tile([HID, EMB], F32, tag=f"wt{gi}")
                nc.vector.tensor_copy(wT[:], wT_ps[:])
                m_ps = pp1.tile([EMB, HID], F32, tag="p1")
                nc.tensor.matmul(m_ps[:], wT[:], blob_sb[0:HID, olg:olg + HID], start=True, stop=True)
                nc.scalar.activation(mzh[:, 64 * gi:64 * (gi + 1)], m_ps[:], AF.Copy, scale=scale)
                bb_ps = pp1.tile([HID, 1], F32, tag="p1")
                nc.tensor.matmul(bb_ps[:], blob_sb[0:HID, olg:olg + HID], bcol(ob), start=True, stop=True)
                bsum = cp.tile([HID, 1], F32, tag=f"bsum{gi}")
                nc.vector.tensor_tensor(out=bsum[:], in0=bb_ps[:], in1=bcol(olb), op=ALU.add)
                bscl = cp.tile([HID, 1], F32, tag=f"bscl{gi}")
                nc.scalar.activation(bscl[:], bsum[:], AF.Copy, scale=scale)
                brt_ps = pp1.tile([1, HID], F32, tag="p1")
                nc.tensor.transpose(brt_ps[:], bscl[:], id128[:HID, :HID])
                nc.vector.tensor_copy(brow[0:1, 64 * gi:64 * (gi + 1)], brt_ps[:])
            brow4 = cp.tile([1, 512], F32)
            for rr in range(4):
                nc.vector.tensor_copy(brow4[0:1, 128 * rr:128 * (rr + 1)], brow[:])
            brows_ps = pp1.tile([128, 512], F32, tag="p1")
            nc.tensor.matmul(brows_ps[:], ones_row[:], brow4[:], start=True, stop=True)
            brows = cp.tile([128, 512], BF16)
            nc.vector.tensor_copy(brows[:], brows_ps[:])

            # ---------------- LOS blend -> pooling matrix Gall --------------
            losf = cp.tile([1, BSH], F32)
            nc.vector.tensor_copy(losf[:], los_sb[:])
            losb_ps = pp1.tile([T, BSH], F32, tag="p1")
            nc.tensor.matmul(losb_ps[:], ones_row[:1, :T], losf[:], start=True, stop=True)
            mask = cp.tile([T, BSH], F32)
            nc.vector.tensor_tensor(out=mask[:], in0=blob_sb[0:T, OIOT:OIOT + 1].to_broadcast([T, BSH]),
                                    in1=losb_ps[:], op=ALU.is_lt)
            ecol = cp.tile([T, 1], F32)
            nc.scalar.activation(ecol[:], blob_sb[0:T, OATT:OATT + 1], AF.Exp)
            esum_ps = pp1.tile([1, 1], F32, tag="p1")
            nc.tensor.matmul(esum_ps[:], ecol[:], ones_col[:T, :], start=True, stop=True)
            rinv = cp.tile([1, 1], F32)
            nc.vector.reciprocal(rinv[:], esum_ps[:])
            wraw_ps = pp1.tile([1, BSH], F32, tag="p1")
            nc.tensor.matmul(wraw_ps[:], ecol[:], mask[:], start=True, stop=True)
            wrow = cp.tile([1, BSH], F32)
            nc.vector.tensor_scalar(out=wrow[:], in0=wraw_ps[:], scalar1=rinv[:, :1],
                                    scalar2=None, op0=ALU.mult)
            w128_ps = pp1.tile([128, BSH], F32, tag="p1")
            nc.tensor.matmul(w128_ps[:], ones_row[:], wrow[:], start=True, stop=True)
            gtmp = cp.tile([128, BSH], F32)
            nc.vector.tensor_tensor(out=gtmp[:], in0=w128_ps[:], in1=blob_sb[:, OPDIF:OPDIF + BSH],
                                    op=ALU.mult)
            gall = cp.tile([128, BSH], BF16)
            nc.vector.tensor_tensor(out=gall[:], in0=gtmp[:], in1=blob_sb[:, OPDIS:OPDIS + BSH],
                                    op=ALU.add)

            # ---------------- phase 1: one-hots -> X^T -> Q^T ----------------
            i100b = cp.tile([V, 1], I8)
            nc.vector.tensor_copy(i100b[:], blob_sb[0:V, OI100:OI100 + 1])
            o_sb = cp.tile([V, R], BF16)
            qt = cp.tile([128, R], BF16)
            qtc = cp.tile([128, R], BF16)

            def p1_a(k):
                # x values replicated as int8: 400KB instead of 1.6MB of DMA
                xrep = wp.tile([V, 512], I8, tag="xrep")
                t = xbf[512 * k:512 * (k + 1)]
                nc.sync.dma_start(out=xrep[:], in_=bass.AP(t.tensor, t.offset, [[0, V]] + list(t.ap)))
                nc.vector.tensor_tensor(out=o_sb[:, 512 * k:512 * (k + 1)],
                                        in0=i100b[:, :1].to_broadcast([V, 512]),
                                        in1=xrep[:], op=ALU.is_equal)
                xt_ps = pp1.tile([EMB, 512], F32, tag="p1")
                for cc in range(8):
                    c = 8 * k + cc
                    nc.tensor.matmul(xt_ps[:, 64 * cc:64 * (cc + 1)],
                                     tp3_sb[:, EMB * c:EMB * (c + 1)],
                                     o_sb[:, 64 * c:64 * (c + 1)], start=True, stop=True)
                xt = wp.tile([EMB, 512], BF16, tag="xt")
                nc.scalar.activation(xt[:], xt_ps[:], AF.Copy)
                return xt

            # qt is stored b-major (col = b*64 + c) so phase-2 transposes can
            # read plain [128, 128] slices; the copy scatters via a strided AP
            qt3v = qt[:].rearrange("p (b c) -> p c b", c=C)

            def p1_b(k, xt):
                q_ps = pp1.tile([128, 512], F32, tag="p1")
                nc.tensor.matmul(q_ps[:], mzh[:], xt[:], start=True, stop=True)
                # the (c,b)->(b,c) scatter costs ~2.5us per 512 cols on any
                # engine; spread the 8 of them across ACT / GPSIMD / DVE
                dst = qt3v[:, 8 * k:8 * (k + 1), :]
                srcv = q_ps[:].rearrange("p (c b) -> p c b", c=8)
                if k in (0, 3, 6):
                    nc.scalar.activation(dst, srcv, AF.Copy)
                elif k in (1, 2, 4, 7):
                    nc.scalar.activation(qtc[:, 512 * k:512 * (k + 1)], q_ps[:], AF.Copy)
                    nc.gpsimd.tensor_copy(
                        dst, qtc[:, 512 * k:512 * (k + 1)].rearrange("p (c b) -> p c b", c=8))
                else:
                    nc.vector.tensor_copy(dst, srcv)

            # phase 1 only needs blob/tp3/xbf - let it win scheduler ties
            # over the adjacency/gate prep chain emitted above
            xts = {}
            with tc.high_priority():
                for k in range(NBATCH + 1):
                    if k < NBATCH:
                        xts[k] = p1_a(k)
                    if k >= 1:
                        p1_b(k - 1, xts.pop(k - 1))

            # ---------------- phase 2: per-chunk transpose/adjacency/gates ---
            pool_ps = ppA.tile([HID, BSH], F32, tag="pA")
            NQUAD = NCHUNK // 4

            def p2_a(p):
                # four chunk transposes into one psum tile
                tr_ps = ppT.tile([128, 512], BF16)
                for h in range(4):
                    nc.tensor.transpose(tr_ps[:, 128 * h:128 * (h + 1)],
                                        qt[:, 512 * p + 128 * h:512 * p + 128 * (h + 1)],
                                        idb[:])
                return tr_ps

            def p2_b(p, tr_ps):
                qr = wp.tile([128, 512], BF16, tag="qr")
                nc.vector.tensor_copy(qr[:], tr_ps[:])
                s_ps = ppS2.tile([128, 512], F32)
                nc.tensor.matmul(s_ps[:], BDb[:], qr[:], start=True, stop=True)
                return s_ps

            def p2_c(p, s_ps):
                sb = wp.tile([128, 512], BF16, tag="sb")
                nc.vector.tensor_tensor(out=sb[:], in0=s_ps[:], in1=brows[:], op=ALU.add)
                u = wp.tile([128, 512], BF16, tag="u")
                nc.scalar.activation(u[:], sb[:], AF.Tanh)
                u4 = u[:].rearrange("q (k g o) -> q k g o", k=4, g=2)
                w1 = wp.tile([128, 4, HID], BF16, tag="w1")
                nc.vector.tensor_tensor(out=w1[:], in0=u4[:, :, 0, :], in1=u4[:, :, 1, :],
                                        op=ALU.mult)
                w2 = wp.tile([128, 4, HID], BF16, tag="w2")
                nc.vector.tensor_tensor(out=w2[:], in0=w1[:], in1=u4[:, :, 1, :], op=ALU.add)
                for h in range(4):
                    j = 4 * p + h
                    nc.tensor.matmul(pool_ps[:, 2 * j:2 * (j + 1)], w2[:, h, :],
                                     gall[:, 2 * j:2 * (j + 1)], start=True, stop=True)

            st_a, st_b = {}, {}
            for p in range(NQUAD + 2):
                if p < NQUAD:
                    st_a[p] = p2_a(p)
                if 1 <= p <= NQUAD:
                    st_b[p - 1] = p2_b(p - 1, st_a.pop(p - 1))
                if 2 <= p:
                    p2_c(p - 2, st_b.pop(p - 2))

            # ---------------- classifier ----------------
            pt = cp.tile([HID, BSH], F32)
            nc.vector.tensor_copy(pt[:], pool_ps[:])
            u1_ps = pp1.tile([2 * HID, BSH], F32, tag="p1")
            nc.tensor.matmul(u1_ps[:], blob_sb[0:HID, OWC1:OWC1 + 2 * HID], pt[:], start=True, stop=True)
            v = cp.tile([2 * HID, BSH], F32)
            nc.scalar.activation(v[:], u1_ps[:], AF.Relu, bias=bcol(OBC1, 128), scale=1.0 / 64.0)
            y_ps2 = pp1.tile([1, BSH], F32, tag="p1")
            nc.tensor.matmul(y_ps2[:], blob_sb[0:128, OWC2:OWC2 + 1], v[:], start=True, stop=True)
            yrow = cp.tile([1, BSH], F32)
            nc.scalar.activation(yrow[:], y_ps2[:], AF.Identity, bias=bcol(OBC2, 1))
            nc.sync.dma_start(out=out[:], in_=yrow[:])

    nc.finalize()
    return nc


def _stage(inputs):
    """Host-side staging: shard + pack.  Pure layout work, no model math."""
    x_batch = np.asarray(inputs["x_batch"]).astype(np.int32)
    los = np.asarray(inputs["LOS_batch"]).astype(np.int32)
    edge = np.asarray(inputs["template_edge_index"]).astype(np.int32)
    emb = np.asarray(inputs["emb_table"], dtype=np.float32)

    tp = np.zeros((C * V, 64), np.float32)
    tp[:, :EMB] = emb.reshape(C * V, EMB)

    blob = np.zeros((128, BLOBF), np.float32)
    blob[0:EMB, OWZ:OWZ + HID] = inputs["W_z"]
    blob[0:EMB, OWH:OWH + HID] = inputs["W_h"]
    blob[0:HID, OLZ:OLZ + HID] = np.asarray(inputs["L_z"])[:HID]
    blob[0:HID, OLH:OLH + HID] = np.asarray(inputs["L_h"])[:HID]
    blob[0:HID, OWC1:OWC1 + 2 * HID] = inputs["Wc1"]
    blob[0:HID, OBZ] = inputs["b_z"]
    blob[0:HID, OBH] = inputs["b_h"]
    blob[0:HID, OLBZ] = inputs["lb_z"]
    blob[0:HID, OLBH] = inputs["lb_h"]
    blob[0:2 * HID, OWC2] = np.asarray(inputs["Wc2"])[:, 0]
    blob[0:2 * HID, OBC1] = inputs["bc1"]
    blob[0, OBC2] = np.asarray(inputs["bc2"])[0]
    blob[0:T, OATT] = inputs["att"]
    blob[:, OID:OID + 128] = np.eye(128, dtype=np.float32)
    blob[0, OION:OION + N] = np.arange(N, dtype=np.float32)
    blob[0:T, OIOT] = np.arange(T, dtype=np.float32)
    blob[0:V, OI100] = np.arange(V, dtype=np.float32)
    # pooling/blend selection patterns: col b, chunk j=b//2, q=b%2
    # ad rows 64q..64q+32, dis rows 64q+32..64q+64
    p = np.arange(128)[:, None]
    b = np.arange(BSH)[None, :]
    p_ad = (p // 32 == 2 * (b % 2)).astype(np.float32)
    p_dis = (p // 32 == 2 * (b % 2) + 1).astype(np.float32)
    blob[:, OPDIF:OPDIF + BSH] = p_ad - p_dis
    blob[:, OPDIS:OPDIS + BSH] = p_dis

    col_off = (np.arange(C, dtype=np.int32) * V)[None, :]
    in_maps = []
    for i in range(NCORES):
        xs = x_batch[i * BSH:(i + 1) * BSH]            # [64, 64]
        flat = (xs + col_off).astype(np.int16).ravel()  # row r = b*64+c
        wrapped = np.tile(flat.reshape(R // 16, 16).T, (8, 1)).copy()  # [128, R//16]
        in_maps.append({
            "tp": tp,
            "gidx": wrapped,
            "edge": edge,
            "los": los[i * BSH:(i + 1) * BSH].reshape(1, BSH).copy(),
            "blob": blob,
        })
    return in_maps


def _stage_v2(inputs):
    """Host staging for the PE-one-hot kernel: transposed bf16 table +
    x values as f32 in (c-major, b-minor) order."""
    x_batch = np.asarray(inputs["x_batch"]).astype(np.int32)
    emb = np.asarray(inputs["emb_table"], dtype=np.float32)
    base = _stage(inputs)
    # [v, (c, e)] layout, bf16
    import ml_dtypes
    tp3 = np.ascontiguousarray(
        emb.transpose(1, 0, 2).reshape(V, C * EMB)).astype(ml_dtypes.bfloat16)
    in_maps = []
    for i in range(NCORES):
        xs = x_batch[i * BSH:(i + 1) * BSH]                 # [64 b, 64 c]
        xbf = np.ascontiguousarray(xs.T).reshape(-1).astype(np.int8)  # c-major
        m = {k: base[i][k] for k in ("edge", "los", "blob")}
        m["tp3"] = tp3
        m["xbf"] = xbf
        in_maps.append(m)
    return in_maps


def kernel(**inputs) -> np.ndarray:
    global LAST_EXEC_NS
    ver = os.environ.get("BASSKERNEL_VER", "2")
    if ver not in _CACHE:
        _CACHE[ver] = _build_nc_v2() if ver == "2" else _build_nc()
    nc = _CACHE[ver]
    in_maps = _stage_v2(inputs) if ver == "2" else _stage(inputs)
    trace = bool(int(os.environ.get("BASSKERNEL_TRACE", "0")))
    kw = {}
    if trace:
        _install_ntff_hook()
        kw["trace"] = True
        tmpdir = os.environ.get("BASSKERNEL_TMPDIR")
        if tmpdir:
            kw["tmpdir"] = tmpdir
    res = run_bass_kernel_spmd(nc, in_maps, core_ids=list(range(NCORES)), **kw)
    LAST_EXEC_NS = getattr(res, "exec_time_ns", None)
    out = np.empty((B, 1), np.float32)
    for i in range(NCORES):
        out[i * BSH:(i + 1) * BSH, 0] = np.asarray(res.results[i]["out"]).reshape(BSH)
    return out


# revision 38
# speedup vs baseline: 1.1675x; 1.0076x over previous
"""Trainium2 Bass kernel for the A3TGCN-2-points model (8 NeuronCores, data-parallel).

Math (verified vs a line-by-line port of the reference at 3.5e-8):
  - The reference passes H=None each period, so H0 = 0: the reset gate R
    vanishes and only the first HID rows of L_z / L_h matter.
  - x_temporal takes two values per sample (admission cols before t < LOS,
    discharge after), so the 37-step attention scan collapses to
        H = w * cell(ad) + (1 - w) * cell(dis),  w = cumsum(softmax(att))[LOS]
    cell(X) = (1 - sigmoid(A X Wz Lz1 + beta_z)) * tanh(A X Wh Lh1 + beta_h)
  - 1 - sigmoid(s) = (1 - tanh(s/2)) / 2, so ONE tanh evaluates both gates
    (the z columns of the fused weights are scaled by -1/2).

Sharding: batch 512 -> 64 samples per core; all weights replicated; the
[64, 1] logits per core are concatenated on the host.

Device mapping, default version 2 (BASSKERNEL_VER=1 selects the dma_gather
variant):
  - embedding lookup via PE one-hot matmuls from a host-transposed [v,(c,e)]
    table (one [100,32]x[100,64] matmul per column) -> X^T, e on partitions
  - fused gate weights Mzh = [-Wz@Lz1/2 | Wh@Lh1] applied as one matmul per
    512 columns -> Q^T, then a (c,b)->(b,c) free-axis reorder (split across
    ACT/GPSIMD/DVE - it costs ~2.5us/512 cols on any single engine)
  - per 128-row chunk: PE transpose, adjacency I4 (x) A^T matmul (gcn_norm
    built on device from edge_index via one-hot matmuls), bias add, tanh,
    gate combine, and pooling+LOS-blend fused into a [128,64]x[128,2] matmul
  - classifier MLP on the pooled [64, 64] tile, 64 f32 out per core.
"""

import os
import sys

import numpy as np

sys.path.insert(0, "/opt/trn_rl_repo")

import concourse.bacc as bacc
import concourse.bass as bass
import concourse.mybir as mybir
import concourse.tile as tile
from concourse.bass_utils import run_bass_kernel_spmd

F32 = mybir.dt.float32
I32 = mybir.dt.int32
I16 = mybir.dt.int16
I8 = mybir.dt.int8
BF16 = mybir.dt.bfloat16
AF = mybir.ActivationFunctionType
ALU = mybir.AluOpType

B, C, N, V, EMB, HID, E, T = 512, 64, 32, 100, 32, 64, 256, 37
NCORES = 8
BSH = B // NCORES            # samples per core
R = BSH * C                  # gathered rows per core (4096)
NCHUNK = R // 128            # 32 row-chunks of 128
NBATCH = NCHUNK // 4         # 8 batches of 4 chunks ([*, 512] tiles)

# blob column layout (weights packed into one [128, 392] f32 DMA)
OWZ, OWH, OLZ, OLH, OWC1 = 0, 64, 128, 192, 256
OBZ, OBH, OLBZ, OLBH, OWC2, OBC1, OBC2, OATT = 384, 385, 386, 387, 388, 389, 390, 391
OID, OION, OIOT = 392, 520, 552
OI100, OPDIF, OPDIS = 553, 554, 618
BLOBF = 682


def _install_ntff_hook():
    """The agent image's antenv lacks axon_hooks; synthesize it so trace=True
    can drive NTFF profiling via ctypes on libaxon_pjrt.so (mirrors the
    boot-side hook in trn_boot.py)."""
    import contextlib
    import ctypes
    import types

    if "antenv.axon_hooks" in sys.modules:
        return
    so_path = "/opt/axon/libaxon_pjrt.so"
    mod = types.ModuleType("antenv.axon_hooks")
    state = {"hook": None}

    def set_axon_ntff_profile_hook(h):
        state["hook"] = h

    def get_axon_ntff_profile_hook():
        return state["hook"]

    mod.set_axon_ntff_profile_hook = set_axon_ntff_profile_hook
    mod.get_axon_ntff_profile_hook = get_axon_ntff_profile_hook
    sys.modules["antenv.axon_hooks"] = mod
    try:
        import antenv
        antenv.axon_hooks = mod
    except ImportError:
        pass

    if not os.path.exists(so_path):
        return
    lib = ctypes.CDLL(so_path)
    if not hasattr(lib, "axon_start_nrt_profile"):
        return
    lib.axon_start_nrt_profile.argtypes = [ctypes.POINTER(ctypes.c_int64), ctypes.c_size_t]
    lib.axon_start_nrt_profile.restype = ctypes.c_int64
    lib.axon_stop_nrt_profile.argtypes = [ctypes.c_char_p]
    lib.axon_stop_nrt_profile.restype = ctypes.c_int64

    @contextlib.contextmanager
    def _hook(output_dir, device_ids):
        import jax
        jax.devices()
        if device_ids:
            ids = (ctypes.c_int64 * len(device_ids))(*device_ids)
            rc = lib.axon_start_nrt_profile(ids, len(device_ids))
        else:
            rc = lib.axon_start_nrt_profile(None, 0)
        if rc != 0:
            raise RuntimeError(f"axon_start_nrt_profile rc={rc}")
        try:
            yield
        finally:
            n = lib.axon_stop_nrt_profile(str(output_dir).encode())
            print(f"profile: {n} file(s) written to {output_dir}", file=sys.stderr)

    set_axon_ntff_profile_hook(_hook)


_CACHE = {}
LAST_EXEC_NS = None


def _build_nc():
    nc = bacc.Bacc("TRN2")

    tp = nc.declare_dram_parameter("tp", [C * V, 64], F32, isOutput=False)
    gidx = nc.declare_dram_parameter("gidx", [128, R // 16], I16, isOutput=False)
    edge = nc.declare_dram_parameter("edge", [2, E], I32, isOutput=False)
    los = nc.declare_dram_parameter("los", [1, BSH], I32, isOutput=False)
    blob = nc.declare_dram_parameter("blob", [128, BLOBF], F32, isOutput=False)
    out = nc.declare_dram_parameter("out", [1, BSH], F32, isOutput=True)

    with tile.TileContext(nc) as tc:
        with (
            tc.tile_pool(name="const", bufs=1) as cp,
            tc.tile_pool(name="work", bufs=3) as wp,
            tc.tile_pool(name="ppY", bufs=3, space="PSUM") as ppY,
            tc.tile_pool(name="ppS", bufs=2, space="PSUM") as ppS,
            tc.tile_pool(name="ppA", bufs=1, space="PSUM") as ppA,
        ):
            # ---------------- input DMAs ----------------
            gsb = cp.tile([128, R // 16], I16)
            nc.sync.dma_start(out=gsb[:], in_=gidx[:])
            # dummy 16-row dma_gather: forces the Q7 mlp library load to
            # overlap the input DMAs instead of delaying the first real gather
            warm_idx = cp.tile([128, 1], I16)
            nc.vector.memset(warm_idx[:], 0)
            warm_out = cp.tile([128, 1, 64], F32)
            nc.gpsimd.dma_gather(
                out_ap=warm_out[:], in_ap=tp[:], idxs_ap=warm_idx[:],
                num_idxs=16, num_idxs_reg=16, elem_size=64)
            blob_sb = cp.tile([128, BLOBF], F32)
            nc.sync.dma_start(out=blob_sb[:], in_=blob[:])
            esrc = cp.tile([128, 2], I32)
            nc.sync.dma_start(out=esrc[:], in_=edge[0].rearrange("(k p) -> p k", p=128))
            edst = cp.tile([128, 2], I32)
            nc.sync.dma_start(out=edst[:], in_=edge[1].rearrange("(k p) -> p k", p=128))
            los_sb = cp.tile([1, BSH], I32)
            nc.sync.dma_start(out=los_sb[:], in_=los[:])

            def bcol(off, rows=64):
                return blob_sb[0:rows, off:off + 1]

            # ---------------- embedding gather ----------------
            # the SWDGE descriptor ring tops out between 1k and 2k entries per
            # shot; 4 gathers of 1024 rows, interleaved with the batches that
            # consume them (emitted in the main loop below)
            xg = cp.tile([128, NCHUNK, 64], F32)
            GCH = 1024

            def issue_gather(c0, c1):
                # gathers rows for chunks [c0, c1)
                nc.gpsimd.dma_gather(
                    out_ap=xg[:, c0:c1, :],
                    in_ap=tp[:],
                    idxs_ap=gsb[:, 8 * c0:8 * c1],
                    num_idxs=128 * (c1 - c0),
                    num_idxs_reg=128 * (c1 - c0),
                    elem_size=64,
                )

            # ---------------- constants ----------------
            id128 = blob_sb[:, OID:OID + 128]
            ones_col = cp.tile([128, 1], F32)
            nc.vector.memset(ones_col[:], 1.0)
            ones_row = cp.tile([1, 128], F32)
            nc.vector.memset(ones_row[:], 1.0)
            iota_nf = cp.tile([128, N], F32)
            _src = blob[0, OION:OION + N]
            nc.sync.dma_start(out=iota_nf[:], in_=bass.AP(_src.tensor, _src.offset, [[0, 128]] + list(_src.ap)))

            # ---------------- adjacency build: BD = I4 (x) A^T ----------------
            srcf = cp.tile([128, 2], F32)
            nc.vector.tensor_copy(srcf[:], esrc[:])
            dstf = cp.tile([128, 2], F32)
            nc.vector.tensor_copy(dstf[:], edst[:])

            Dk, Sk = [], []
            for k in range(2):
                d = cp.tile([128, N], F32, tag=f"dk{k}")
                nc.vector.tensor_tensor(
                    out=d[:], in0=dstf[:, k:k + 1].to_broadcast([128, N]),
                    in1=iota_nf[:], op=ALU.is_equal)
                s = cp.tile([128, N], F32, tag=f"sk{k}")
                nc.vector.tensor_tensor(
                    out=s[:], in0=srcf[:, k:k + 1].to_broadcast([128, N]),
                    in1=iota_nf[:], op=ALU.is_equal)
                Dk.append(d)
                Sk.append(s)

            deg_ps = ppS.tile([1, N], F32, tag="s_ps")
            nc.tensor.matmul(deg_ps[:], ones_col[:], Dk[0][:], start=True, stop=False)
            nc.tensor.matmul(deg_ps[:], ones_col[:], Dk[1][:], start=False, stop=True)
            degp1 = cp.tile([1, N], F32)
            nc.scalar.activation(degp1[:], deg_ps[:], AF.Identity, bias=1.0)
            rec = cp.tile([1, N], F32)
            nc.vector.reciprocal(rec[:], degp1[:])
            dinv_row = cp.tile([1, N], F32)
            nc.scalar.activation(dinv_row[:], rec[:], AF.Sqrt)

            dinvb_ps = ppS.tile([128, N], F32, tag="s_ps")
            nc.tensor.matmul(dinvb_ps[:], ones_row[:], dinv_row[:], start=True, stop=True)
            dinvb = cp.tile([128, N], F32)
            nc.vector.tensor_copy(dinvb[:], dinvb_ps[:])

            at_ps = ppA.tile([N, N], F32)
            for k in range(2):
                tmp = cp.tile([128, N], F32, tag="degtmp")
                nc.vector.tensor_tensor(out=tmp[:], in0=Dk[k][:], in1=dinvb[:], op=ALU.mult)
                dd = cp.tile([128, 1], F32, tag="ddk")
                nc.vector.tensor_reduce(dd[:], tmp[:], axis=mybir.AxisListType.X, op=ALU.add)
                nc.vector.tensor_tensor(out=tmp[:], in0=Sk[k][:], in1=dinvb[:], op=ALU.mult)
                ds_ = cp.tile([128, 1], F32, tag="dsk")
                nc.vector.tensor_reduce(ds_[:], tmp[:], axis=mybir.AxisListType.X, op=ALU.add)
                nrm = cp.tile([128, 1], F32, tag="nrmk")
                nc.vector.tensor_tensor(out=nrm[:], in0=dd[:], in1=ds_[:], op=ALU.mult)
                sn = cp.tile([128, N], F32, tag=f"snk{k}")
                nc.vector.tensor_scalar(out=sn[:], in0=Sk[k][:], scalar1=nrm[:, :1],
                                        scalar2=None, op0=ALU.mult)
                nc.tensor.matmul(at_ps[:], sn[:], Dk[k][:], start=(k == 0), stop=False)
            diagd = cp.tile([N, N], F32)
            nc.vector.tensor_tensor(out=diagd[:], in0=id128[:N, :N], in1=dinvb[:N, :],
                                    op=ALU.mult)
            nc.tensor.matmul(at_ps[:], diagd[:], diagd[:], start=False, stop=True)

            # engines are lane-locked (no partition shifts), so place the four
            # diagonal blocks with SBUF->SBUF DMAs
            at_sb = cp.tile([N, N], F32)
            nc.vector.tensor_copy(at_sb[:], at_ps[:])
            BD = cp.tile([128, 128], F32)
            nc.vector.memset(BD[:], 0.0)
            for q in range(4):
                nc.sync.dma_start(out=BD[32 * q:32 * (q + 1), 32 * q:32 * (q + 1)],
                                  in_=at_sb[:])

            # ---------------- fused gate weights Mzh = [-Mz/2 | Mh] ----------------
            mzh = cp.tile([EMB, 128], F32)
            betas = []
            for gi, (ow, ob, olb, olg, scale) in enumerate((
                    (OWZ, OBZ, OLBZ, OLZ, -0.5), (OWH, OBH, OLBH, OLH, 1.0))):
                wT_ps = ppS.tile([HID, EMB], F32, tag="s_ps")
                nc.tensor.transpose(wT_ps[:], blob_sb[0:EMB, ow:ow + HID], id128[:EMB, :EMB])
                wT = cp.tile([HID, EMB], F32, tag=f"wt{gi}")
                nc.vector.tensor_copy(wT[:], wT_ps[:])
                m_ps = ppS.tile([EMB, HID], F32, tag="s_ps")
                nc.tensor.matmul(m_ps[:], wT[:], blob_sb[0:HID, olg:olg + HID],
                                 start=True, stop=True)
                nc.scalar.activation(mzh[:, 64 * gi:64 * (gi + 1)], m_ps[:], AF.Copy,
                                     scale=scale)
                # beta_g = Lg1^T b_g + lb_g  (as a column), scaled like Mz/Mh
                bb_ps = ppS.tile([HID, 1], F32, tag="s_ps")
                nc.tensor.matmul(bb_ps[:], blob_sb[0:HID, olg:olg + HID], bcol(ob),
                                 start=True, stop=True)
                bsum = cp.tile([HID, 1], F32, tag=f"bsum{gi}")
                nc.vector.tensor_tensor(out=bsum[:], in0=bb_ps[:], in1=bcol(olb), op=ALU.add)
                bcolg = cp.tile([HID, 1], F32, tag=f"beta{gi}")
                nc.scalar.activation(bcolg[:], bsum[:], AF.Copy, scale=scale)
                betas.append(bcolg)

            # ---------------- LOS blend weights ----------------
            losf = cp.tile([1, BSH], F32)
            nc.vector.tensor_copy(losf[:], los_sb[:])
            losb_ps = ppS.tile([T, BSH], F32, tag="s_ps")
            nc.tensor.matmul(losb_ps[:], ones_row[:1, :T], losf[:], start=True, stop=True)
            mask = cp.tile([T, BSH], F32)
            nc.vector.tensor_tensor(out=mask[:], in0=blob_sb[0:T, OIOT:OIOT + 1].to_broadcast([T, BSH]),
                                    in1=losb_ps[:], op=ALU.is_lt)
            ecol = cp.tile([T, 1], F32)
            nc.scalar.activation(ecol[:], blob_sb[0:T, OATT:OATT + 1], AF.Exp)
            tanh_warm = cp.tile([1, 1], F32)
            nc.scalar.activation(tanh_warm[:], ones_col[0:1, 0:1], AF.Tanh)
            esum_ps = ppS.tile([1, 1], F32, tag="s_ps")
            nc.tensor.matmul(esum_ps[:], ecol[:], ones_col[:T, :], start=True, stop=True)
            rinv = cp.tile([1, 1], F32)
            nc.vector.reciprocal(rinv[:], esum_ps[:])
            wraw_ps = ppS.tile([1, BSH], F32, tag="s_ps")
            nc.tensor.matmul(wraw_ps[:], ecol[:], mask[:], start=True, stop=True)
            wrow = cp.tile([1, BSH], F32)
            nc.vector.tensor_scalar(out=wrow[:], in0=wraw_ps[:], scalar1=rinv[:, :1],
                                    scalar2=None, op0=ALU.mult)
            wb_ps = ppS.tile([HID, BSH], F32, tag="s_ps")
            nc.tensor.matmul(wb_ps[:], ones_row[:1, :HID], wrow[:], start=True, stop=True)
            wb = cp.tile([HID, BSH], F32)
            nc.vector.tensor_copy(wb[:], wb_ps[:])

            # ---------------- main loop ----------------
            sums = cp.tile([HID, 2 * BSH], F32)
            gather_plan = {0: (0, 8), 2: (8, 16), 4: (16, 24), 6: (24, 28), 7: (28, 32)}
            for jb in range(NBATCH):
                if jb in gather_plan:
                    issue_gather(*gather_plan[jb])
                y_ps = ppY.tile([EMB, 512], F32)
                for jj in range(4):
                    j = 4 * jb + jj
                    nc.tensor.matmul(y_ps[:, 128 * jj:128 * (jj + 1)],
                                     xg[:, j, 0:EMB], BD[:], start=True, stop=True)
                ysb = wp.tile([EMB, 512], F32)
                nc.vector.tensor_copy(ysb[:], y_ps[:])
                # z- and h-gate pre-activations side by side on the SAME
                # partitions (engines cannot shift lanes)
                s_ps = ppS.tile([HID, 1024], F32, tag="s_ps")
                nc.tensor.matmul(s_ps[:, 0:512], mzh[:, 0:64], ysb[:],
                                 start=True, stop=True)
                nc.tensor.matmul(s_ps[:, 512:1024], mzh[:, 64:128], ysb[:],
                                 start=True, stop=True)
                u = wp.tile([HID, 1024], BF16)
                nc.scalar.activation(u[:, 0:512], s_ps[:, 0:512], AF.Tanh,
                                     bias=betas[0][:, :1])
                nc.scalar.activation(u[:, 512:1024], s_ps[:, 512:1024], AF.Tanh,
                                     bias=betas[1][:, :1])
                w1 = wp.tile([HID, 512], BF16)
                nc.vector.tensor_tensor(out=w1[:], in0=u[:, 0:512], in1=u[:, 512:1024],
                                        op=ALU.mult)
                w2 = wp.tile([HID, 512], BF16)
                nc.vector.tensor_tensor(out=w2[:], in0=w1[:], in1=u[:, 512:1024],
                                        op=ALU.add)
                nc.vector.tensor_reduce(
                    sums[:, 16 * jb:16 * (jb + 1)],
                    w2[:].rearrange("p (g n) -> p g n", n=N),
                    axis=mybir.AxisListType.X, op=ALU.add)

            # ---------------- blend + pool + classifier ----------------
            s3 = sums[:].rearrange("p (s k) -> p s k", k=2)
            t1 = cp.tile([HID, BSH], F32)
            nc.vector.tensor_tensor(out=t1[:], in0=s3[:, :, 0], in1=s3[:, :, 1],
                                    op=ALU.subtract)
            t2 = cp.tile([HID, BSH], F32)
            nc.vector.tensor_tensor(out=t2[:], in0=t1[:], in1=wb[:], op=ALU.mult)
            pt = cp.tile([HID, BSH], F32)
            nc.vector.tensor_tensor(out=pt[:], in0=t2[:], in1=s3[:, :, 1], op=ALU.add)

            u1_ps = ppS.tile([2 * HID, BSH], F32, tag="s_ps")
            nc.tensor.matmul(u1_ps[:], blob_sb[0:HID, OWC1:OWC1 + 2 * HID], pt[:],
                             start=True, stop=True)
            v = cp.tile([2 * HID, BSH], F32)
            nc.scalar.activation(v[:], u1_ps[:], AF.Relu, bias=bcol(OBC1, 128),
                                 scale=1.0 / 64.0)
            y_ps2 = ppS.tile([1, BSH], F32, tag="s_ps")
            nc.tensor.matmul(y_ps2[:], blob_sb[0:128, OWC2:OWC2 + 1], v[:],
                             start=True, stop=True)
            yrow = cp.tile([1, BSH], F32)
            nc.scalar.activation(yrow[:], y_ps2[:], AF.Identity, bias=bcol(OBC2, 1))
            nc.sync.dma_start(out=out[:], in_=yrow[:])

    nc.finalize()
    return nc



def _build_nc_v2():
    """PE-one-hot variant: no GPSIMD at all (no library load, no descriptor
    prep).  Embedding lookup = per-column one-hot matmuls from a transposed
    bf16 table; adjacency applied on transposed row-chunks; pooling and the
    LOS blend fused into a per-chunk matmul."""
    nc = bacc.Bacc("TRN2")

    tp3 = nc.declare_dram_parameter("tp3", [V, C * EMB], BF16, isOutput=False)
    xbf = nc.declare_dram_parameter("xbf", [R], I8, isOutput=False)
    edge = nc.declare_dram_parameter("edge", [2, E], I32, isOutput=False)
    los = nc.declare_dram_parameter("los", [1, BSH], I32, isOutput=False)
    blob = nc.declare_dram_parameter("blob", [128, BLOBF], F32, isOutput=False)
    out = nc.declare_dram_parameter("out", [1, BSH], F32, isOutput=True)

    with tile.TileContext(nc) as tc:
        with (
            tc.tile_pool(name="const", bufs=1) as cp,
            tc.tile_pool(name="work", bufs=3) as wp,
            tc.tile_pool(name="pp1", bufs=3, space="PSUM") as pp1,
            tc.tile_pool(name="ppT", bufs=2, space="PSUM") as ppT,
            tc.tile_pool(name="ppS2", bufs=2, space="PSUM") as ppS2,
            tc.tile_pool(name="ppA", bufs=1, space="PSUM") as ppA,
        ):
            # ---------------- input DMAs ----------------
            blob_sb = cp.tile([128, BLOBF], F32)
            nc.sync.dma_start(out=blob_sb[:], in_=blob[:])
            tp3_sb = cp.tile([V, C * EMB], BF16)
            nc.sync.dma_start(out=tp3_sb[:], in_=tp3[:])
            esrc = cp.tile([128, 2], I32)
            nc.sync.dma_start(out=esrc[:], in_=edge[0].rearrange("(k p) -> p k", p=128))
            edst = cp.tile([128, 2], I32)
            nc.sync.dma_start(out=edst[:], in_=edge[1].rearrange("(k p) -> p k", p=128))
            los_sb = cp.tile([1, BSH], I32)
            nc.sync.dma_start(out=los_sb[:], in_=los[:])

            def bcol(off, rows=64):
                return blob_sb[0:rows, off:off + 1]

            id128 = blob_sb[:, OID:OID + 128]
            ones_col = cp.tile([128, 1], F32)
            nc.vector.memset(ones_col[:], 1.0)
            ones_row = cp.tile([1, 128], F32)
            nc.vector.memset(ones_row[:], 1.0)
            iota_nf = cp.tile([128, N], F32)
            _src = blob[0, OION:OION + N]
            nc.sync.dma_start(out=iota_nf[:], in_=bass.AP(_src.tensor, _src.offset, [[0, 128]] + list(_src.ap)))
            idb = cp.tile([128, 128], BF16)
            nc.scalar.activation(idb[:], id128, AF.Copy)

            # ---------------- adjacency: BD = I4 (x) A^T  (f32 + bf16) -----
            srcf = cp.tile([128, 2], F32)
            nc.vector.tensor_copy(srcf[:], esrc[:])
            dstf = cp.tile([128, 2], F32)
            nc.vector.tensor_copy(dstf[:], edst[:])
            Dk, Sk = [], []
            for k in range(2):
                d = cp.tile([128, N], F32, tag=f"dk{k}")
                nc.vector.tensor_tensor(out=d[:], in0=dstf[:, k:k + 1].to_broadcast([128, N]),
                                        in1=iota_nf[:], op=ALU.is_equal)
                s = cp.tile([128, N], F32, tag=f"sk{k}")
                nc.vector.tensor_tensor(out=s[:], in0=srcf[:, k:k + 1].to_broadcast([128, N]),
                                        in1=iota_nf[:], op=ALU.is_equal)
                Dk.append(d)
                Sk.append(s)
            deg_ps = pp1.tile([1, N], F32, tag="p1")
            nc.tensor.matmul(deg_ps[:], ones_col[:], Dk[0][:], start=True, stop=False)
            nc.tensor.matmul(deg_ps[:], ones_col[:], Dk[1][:], start=False, stop=True)
            degp1 = cp.tile([1, N], F32)
            nc.scalar.activation(degp1[:], deg_ps[:], AF.Identity, bias=1.0)
            rec = cp.tile([1, N], F32)
            nc.vector.reciprocal(rec[:], degp1[:])
            dinv_row = cp.tile([1, N], F32)
            nc.scalar.activation(dinv_row[:], rec[:], AF.Sqrt)
            dinvb_ps = pp1.tile([128, N], F32, tag="p1")
            nc.tensor.matmul(dinvb_ps[:], ones_row[:], dinv_row[:], start=True, stop=True)
            dinvb = cp.tile([128, N], F32)
            nc.vector.tensor_copy(dinvb[:], dinvb_ps[:])
            at_ps = ppA.tile([N, N], F32, tag="pA")
            for k in range(2):
                tmp = cp.tile([128, N], F32, tag="degtmp")
                nc.vector.tensor_tensor(out=tmp[:], in0=Dk[k][:], in1=dinvb[:], op=ALU.mult)
                dd = cp.tile([128, 1], F32, tag="ddk")
                nc.vector.tensor_reduce(dd[:], tmp[:], axis=mybir.AxisListType.X, op=ALU.add)
                nc.vector.tensor_tensor(out=tmp[:], in0=Sk[k][:], in1=dinvb[:], op=ALU.mult)
                ds_ = cp.tile([128, 1], F32, tag="dsk")
                nc.vector.tensor_reduce(ds_[:], tmp[:], axis=mybir.AxisListType.X, op=ALU.add)
                nrm = cp.tile([128, 1], F32, tag="nrmk")
                nc.vector.tensor_tensor(out=nrm[:], in0=dd[:], in1=ds_[:], op=ALU.mult)
                sn = cp.tile([128, N], F32, tag=f"snk{k}")
                nc.vector.tensor_scalar(out=sn[:], in0=Sk[k][:], scalar1=nrm[:, :1],
                                        scalar2=None, op0=ALU.mult)
                nc.tensor.matmul(at_ps[:], sn[:], Dk[k][:], start=(k == 0), stop=False)
            diagd = cp.tile([N, N], F32)
            nc.vector.tensor_tensor(out=diagd[:], in0=id128[:N, :N], in1=dinvb[:N, :], op=ALU.mult)
            nc.tensor.matmul(at_ps[:], diagd[:], diagd[:], start=False, stop=True)
            at_sb = cp.tile([N, N], BF16)
            nc.vector.tensor_copy(at_sb[:], at_ps[:])
            BDb = cp.tile([128, 128], BF16)
            nc.vector.memset(BDb[:], 0.0)
            for q in range(4):
                nc.sync.dma_start(out=BDb[32 * q:32 * (q + 1), 32 * q:32 * (q + 1)], in_=at_sb[:])

            # ---------------- fused gate weights + beta row ----------------
            mzh = cp.tile([EMB, 128], BF16)
            brow = cp.tile([1, 128], F32)
            for gi, (ow, ob, olb, olg, scale) in enumerate((
                    (OWZ, OBZ, OLBZ, OLZ, -0.5), (OWH, OBH, OLBH, OLH, 1.0))):
                wT_ps = pp1.tile([HID, EMB], F32, tag="p1")
                nc.tensor.transpose(wT_ps[:], blob_sb[0:EMB, ow:ow + HID], id128[:EMB, :EMB])
                wT = cp.tile([HID, EMB], F32, tag=f"wt{gi}")
                nc.vector.tensor_copy(wT[:], wT_ps[:])
                m_ps = pp1.tile([EMB, HID], F32, tag="p1")
                nc.tensor.matmul(m_ps[:], wT[:], blob_sb[0:HID, olg:olg + HID], start=True, stop=True)
                nc.scalar.activation(mzh[:, 64 * gi:64 * (gi + 1)], m_ps[:], AF.Copy, scale=scale)
                bb_ps = pp1.tile([HID, 1], F32, tag="p1")
                nc.tensor.matmul(bb_ps[:], blob_sb[0:HID, olg:olg + HID], bcol(ob), start=True, stop=True)
                bsum = cp.tile([HID, 1], F32, tag=f"bsum{gi}")
                nc.vector.tensor_tensor(out=bsum[:], in0=bb_ps[:], in1=bcol(olb), op=ALU.add)
                bscl = cp.tile([HID, 1], F32, tag=f"bscl{gi}")
                nc.scalar.activation(bscl[:], bsum[:], AF.Copy, scale=scale)
                brt_ps = pp1.tile([1, HID], F32, tag="p1")
                nc.tensor.transpose(brt_ps[:], bscl[:], id128[:HID, :HID])
                nc.vector.tensor_copy(brow[0:1, 64 * gi:64 * (gi + 1)], brt_ps[:])
            brow4 = cp.tile([1, 512], F32)
            for rr in range(4):
                nc.vector.tensor_copy(brow4[0:1, 128 * rr:128 * (rr + 1)], brow[:])
            brows_ps = pp1.tile([128, 512], F32, tag="p1")
            nc.tensor.matmul(brows_ps[:], ones_row[:], brow4[:], start=True, stop=True)
            brows = cp.tile([128, 512], BF16)
            nc.vector.tensor_copy(brows[:], brows_ps[:])

            # ---------------- LOS blend -> pooling matrix Gall --------------
            losf = cp.tile([1, BSH], F32)
            nc.vector.tensor_copy(losf[:], los_sb[:])
            losb_ps = pp1.tile([T, BSH], F32, tag="p1")
            nc.tensor.matmul(losb_ps[:], ones_row[:1, :T], losf[:], start=True, stop=True)
            mask = cp.tile([T, BSH], F32)
            nc.vector.tensor_tensor(out=mask[:], in0=blob_sb[0:T, OIOT:OIOT + 1].to_broadcast([T, BSH]),
                                    in1=losb_ps[:], op=ALU.is_lt)
            ecol = cp.tile([T, 1], F32)
            nc.scalar.activation(ecol[:], blob_sb[0:T, OATT:OATT + 1], AF.Exp)
            tanh_warm = cp.tile([1, 1], F32)
            nc.scalar.activation(tanh_warm[:], ones_col[0:1, 0:1], AF.Tanh)
            esum_ps = pp1.tile([1, 1], F32, tag="p1")
            nc.tensor.matmul(esum_ps[:], ecol[:], ones_col[:T, :], start=True, stop=True)
            rinv = cp.tile([1, 1], F32)
            nc.vector.reciprocal(rinv[:], esum_ps[:])
            wraw_ps = pp1.tile([1, BSH], F32, tag="p1")
            nc.tensor.matmul(wraw_ps[:], ecol[:], mask[:], start=True, stop=True)
            wrow = cp.tile([1, BSH], F32)
            nc.vector.tensor_scalar(out=wrow[:], in0=wraw_ps[:], scalar1=rinv[:, :1],
                                    scalar2=None, op0=ALU.mult)
            w128_ps = pp1.tile([128, BSH], F32, tag="p1")
            nc.tensor.matmul(w128_ps[:], ones_row[:], wrow[:], start=True, stop=True)
            gtmp = cp.tile([128, BSH], F32)
            nc.vector.tensor_tensor(out=gtmp[:], in0=w128_ps[:], in1=blob_sb[:, OPDIF:OPDIF + BSH],
                                    op=ALU.mult)
            gall = cp.tile([128, BSH], BF16)
            nc.vector.tensor_tensor(out=gall[:], in0=gtmp[:], in1=blob_sb[:, OPDIS:OPDIS + BSH],
                                    op=ALU.add)

            # ---------------- phase 1: one-hots -> X^T -> Q^T ----------------
            i100b = cp.tile([V, 1], I8)
            nc.vector.tensor_copy(i100b[:], blob_sb[0:V, OI100:OI100 + 1])
            o_sb = cp.tile([V, R], BF16)
            qt = cp.tile([128, R], BF16)
            qtc = cp.tile([128, R], BF16)

            def p1_a(k):
                # x values replicated as int8: 400KB instead of 1.6MB of DMA
                xrep = wp.tile([V, 512], I8, tag="xrep")
                t = xbf[512 * k:512 * (k + 1)]
                nc.sync.dma_start(out=xrep[:], in_=bass.AP(t.tensor, t.offset, [[0, V]] + list(t.ap)))
                nc.vector.tensor_tensor(out=o_sb[:, 512 * k:512 * (k + 1)],
                                        in0=i100b[:, :1].to_broadcast([V, 512]),
                                        in1=xrep[:], op=ALU.is_equal)
                xt_ps = pp1.tile([EMB, 512], F32, tag="p1")
                for cc in range(8):
                    c = 8 * k + cc
                    nc.tensor.matmul(xt_ps[:, 64 * cc:64 * (cc + 1)],
                                     tp3_sb[:, EMB * c:EMB * (c + 1)],
                                     o_sb[:, 64 * c:64 * (c + 1)], start=True, stop=True)
                xt = wp.tile([EMB, 512], BF16, tag="xt")
                if k % 3 == 2:
                    nc.vector.tensor_copy(xt[:], xt_ps[:])
                else:
                    nc.scalar.activation(xt[:], xt_ps[:], AF.Copy)
                return xt

            # qt is stored b-major (col = b*64 + c) so phase-2 transposes can
            # read plain [128, 128] slices; the copy scatters via a strided AP
            qt3v = qt[:].rearrange("p (b c) -> p c b", c=C)

            def p1_b(k, xt):
                q_ps = pp1.tile([128, 512], F32, tag="p1")
                nc.tensor.matmul(q_ps[:], mzh[:], xt[:], start=True, stop=True)
                # the (c,b)->(b,c) scatter costs ~2.5us per 512 cols on any
                # engine; spread the 8 of them across ACT / GPSIMD / DVE
                dst = qt3v[:, 8 * k:8 * (k + 1), :]
                srcv = q_ps[:].rearrange("p (c b) -> p c b", c=8)
                if k in (0, 3, 6):
                    nc.scalar.activation(dst, srcv, AF.Copy)
                elif k in (1, 2, 4, 7):
                    nc.scalar.activation(qtc[:, 512 * k:512 * (k + 1)], q_ps[:], AF.Copy)
                    nc.gpsimd.tensor_copy(
                        dst, qtc[:, 512 * k:512 * (k + 1)].rearrange("p (c b) -> p c b", c=8))
                else:
                    nc.vector.tensor_copy(dst, srcv)

            # phase 1 only needs blob/tp3/xbf - let it win scheduler ties
            # over the adjacency/gate prep chain emitted above
            xts = {}
            with tc.high_priority():
                for k in range(NBATCH + 1):
                    if k < NBATCH:
                        xts[k] = p1_a(k)
                    if k >= 1:
                        p1_b(k - 1, xts.pop(k - 1))

            # ---------------- phase 2: per-chunk transpose/adjacency/gates ---
            pool_ps = ppA.tile([HID, BSH], F32, tag="pA")
            NQUAD = NCHUNK // 4

            def p2_a(p):
                # four chunk transposes into one psum tile
                tr_ps = ppT.tile([128, 512], BF16)
                for h in range(4):
                    nc.tensor.transpose(tr_ps[:, 128 * h:128 * (h + 1)],
                                        qt[:, 512 * p + 128 * h:512 * p + 128 * (h + 1)],
                                        idb[:])
                return tr_ps

            def p2_b(p, tr_ps):
                qr = wp.tile([128, 512], BF16, tag="qr")
                nc.vector.tensor_copy(qr[:], tr_ps[:])
                s_ps = ppS2.tile([128, 512], F32)
                nc.tensor.matmul(s_ps[:], BDb[:], qr[:], start=True, stop=True)
                return s_ps

            def p2_c(p, s_ps):
                sb = wp.tile([128, 512], BF16, tag="sb")
                nc.vector.tensor_tensor(out=sb[:], in0=s_ps[:], in1=brows[:], op=ALU.add)
                u = wp.tile([128, 512], BF16, tag="u")
                nc.scalar.activation(u[:], sb[:], AF.Tanh)
                u4 = u[:].rearrange("q (k g o) -> q k g o", k=4, g=2)
                w1 = wp.tile([128, 4, HID], BF16, tag="w1")
                nc.vector.tensor_tensor(out=w1[:], in0=u4[:, :, 0, :], in1=u4[:, :, 1, :],
                                        op=ALU.mult)
                w2 = wp.tile([128, 4, HID], BF16, tag="w2")
                nc.vector.tensor_tensor(out=w2[:], in0=w1[:], in1=u4[:, :, 1, :], op=ALU.add)
                for h in range(4):
                    j = 4 * p + h
                    nc.tensor.matmul(pool_ps[:, 2 * j:2 * (j + 1)], w2[:, h, :],
                                     gall[:, 2 * j:2 * (j + 1)], start=True, stop=True)

            st_a, st_b = {}, {}
            for p in range(NQUAD + 2):
                if p < NQUAD:
                    st_a[p] = p2_a(p)
                if 1 <= p <= NQUAD:
                    st_b[p - 1] = p2_b(p - 1, st_a.pop(p - 1))
                if 2 <= p:
                    p2_c(p - 2, st_b.pop(p - 2))

            # ---------------- classifier ----------------
            pt = cp.tile([HID, BSH], F32)
            nc.vector.tensor_copy(pt[:], pool_ps[:])
            u1_ps = pp1.tile([2 * HID, BSH], F32, tag="p1")
            nc.tensor.matmul(u1_ps[:], blob_sb[0:HID, OWC1:OWC1 + 2 * HID], pt[:], start=True, stop=True)
            v = cp.tile([2 * HID, BSH], F32)
            nc.scalar.activation(v[:], u1_ps[:], AF.Relu, bias=bcol(OBC1, 128), scale=1.0 / 64.0)
            y_ps2 = pp1.tile([1, BSH], F32, tag="p1")
            nc.tensor.matmul(y_ps2[:], blob_sb[0:128, OWC2:OWC2 + 1], v[:], start=True, stop=True)
            yrow = cp.tile([1, BSH], F32)
            nc.scalar.activation(yrow[:], y_ps2[:], AF.Identity, bias=bcol(OBC2, 1))
            nc.sync.dma_start(out=out[:], in_=yrow[:])

    nc.finalize()
    return nc


def _stage(inputs):
    """Host-side staging: shard + pack.  Pure layout work, no model math."""
    x_batch = np.asarray(inputs["x_batch"]).astype(np.int32)
    los = np.asarray(inputs["LOS_batch"]).astype(np.int32)
    edge = np.asarray(inputs["template_edge_index"]).astype(np.int32)
    emb = np.asarray(inputs["emb_table"], dtype=np.float32)

    tp = np.zeros((C * V, 64), np.float32)
    tp[:, :EMB] = emb.reshape(C * V, EMB)

    blob = np.zeros((128, BLOBF), np.float32)
    blob[0:EMB, OWZ:OWZ + HID] = inputs["W_z"]
    blob[0:EMB, OWH:OWH + HID] = inputs["W_h"]
    blob[0:HID, OLZ:OLZ + HID] = np.asarray(inputs["L_z"])[:HID]
    blob[0:HID, OLH:OLH + HID] = np.asarray(inputs["L_h"])[:HID]
    blob[0:HID, OWC1:OWC1 + 2 * HID] = inputs["Wc1"]
    blob[0:HID, OBZ] = inputs["b_z"]
    blob[0:HID, OBH] = inputs["b_h"]
    blob[0:HID, OLBZ] = inputs["lb_z"]
    blob[0:HID, OLBH] = inputs["lb_h"]
    blob[0:2 * HID, OWC2] = np.asarray(inputs["Wc2"])[:, 0]
    blob[0:2 * HID, OBC1] = inputs["bc1"]
    blob[0, OBC2] = np.asarray(inputs["bc2"])[0]
    blob[0:T, OATT] = inputs["att"]
    blob[:, OID:OID + 128] = np.eye(128, dtype=np.float32)
    blob[0, OION:OION + N] = np.arange(N, dtype=np.float32)
    blob[0:T, OIOT] = np.arange(T, dtype=np.float32)
    blob[0:V, OI100] = np.arange(V, dtype=np.float32)
    # pooling/blend selection patterns: col b, chunk j=b//2, q=b%2
    # ad rows 64q..64q+32, dis rows 64q+32..64q+64
    p = np.arange(128)[:, None]
    b = np.arange(BSH)[None, :]
    p_ad = (p // 32 == 2 * (b % 2)).astype(np.float32)
    p_dis = (p // 32 == 2 * (b % 2) + 1).astype(np.float32)
    blob[:, OPDIF:OPDIF + BSH] = p_ad - p_dis
    blob[:, OPDIS:OPDIS + BSH] = p_dis

    col_off = (np.arange(C, dtype=np.int32) * V)[None, :]
    in_maps = []
    for i in range(NCORES):
        xs = x_batch[i * BSH:(i + 1) * BSH]            # [64, 64]
        flat = (xs + col_off).astype(np.int16).ravel()  # row r = b*64+c
        wrapped = np.tile(flat.reshape(R // 16, 16).T, (8, 1)).copy()  # [128, R//16]
        in_maps.append({
            "tp": tp,
            "gidx": wrapped,
            "edge": edge,
            "los": los[i * BSH:(i + 1) * BSH].reshape(1, BSH).copy(),
            "blob": blob,
        })
    return in_maps


def _stage_v2(inputs):
    """Host staging for the PE-one-hot kernel: transposed bf16 table +
    x values as f32 in (c-major, b-minor) order."""
    x_batch = np.asarray(inputs["x_batch"]).astype(np.int32)
    emb = np.asarray(inputs["emb_table"], dtype=np.float32)
    base = _stage(inputs)
    # [v, (c, e)] layout, bf16
    import ml_dtypes
    tp3 = np.ascontiguousarray(
        emb.transpose(1, 0, 2).reshape(V, C * EMB)).astype(ml_dtypes.bfloat16)
    in_maps = []
    for i in range(NCORES):
        xs = x_batch[i * BSH:(i + 1) * BSH]                 # [64 b, 64 c]
        xbf = np.ascontiguousarray(xs.T).reshape(-1).astype(np.int8)  # c-major
        m = {k: base[i][k] for k in ("edge", "los", "blob")}
        m["tp3"] = tp3
        m["xbf"] = xbf
        in_maps.append(m)
    return in_maps


def kernel(**inputs) -> np.ndarray:
    global LAST_EXEC_NS
    ver = os.environ.get("BASSKERNEL_VER", "2")
    if ver not in _CACHE:
        _CACHE[ver] = _build_nc_v2() if ver == "2" else _build_nc()
    nc = _CACHE[ver]
    in_maps = _stage_v2(inputs) if ver == "2" else _stage(inputs)
    trace = bool(int(os.environ.get("BASSKERNEL_TRACE", "0")))
    kw = {}
    if trace:
        _install_ntff_hook()
        kw["trace"] = True
        tmpdir = os.environ.get("BASSKERNEL_TMPDIR")
        if tmpdir:
            kw["tmpdir"] = tmpdir
    res = run_bass_kernel_spmd(nc, in_maps, core_ids=list(range(NCORES)), **kw)
    LAST_EXEC_NS = getattr(res, "exec_time_ns", None)
    out = np.empty((B, 1), np.float32)
    for i in range(NCORES):
        out[i * BSH:(i + 1) * BSH, 0] = np.asarray(res.results[i]["out"]).reshape(BSH)
    return out
